# revision 1
# baseline (speedup 1.0000x reference)
"""Trainium2 Bass kernel for nn_Attention_40492951666725.

Full attention layer: qkv proj -> RoPE (interleaved pairs, rot dim 32) ->
softmax(QK^T)V -> out proj.  B=4, N=2048, DIM=1024, H=16, DH=64.

Sharding: 8 cores, core c handles batch b=c//2 and query-half c%2 (1024
query tokens, all 16 heads, full 2048-token K/V).  K/V projection is
computed redundantly by the two cores sharing a batch; no collectives.
The host rotates the token axis per core so the core's own query tokens
are always columns [0:1024] of xT (attention is permutation-invariant
over keys, so k/v/cos/sin just follow the same order).

Layouts (per core):
  xT   [DIM, 2048]  (host-transposed)   -> lhsT/rhs for projections
  q^T  [feat, 1024], k^T [feat, 2048]   feat on partitions
  S^T  [kj, qi]  (kj on partitions)     -> softmax via exp (no max-sub;
        scores are O(+-10) so fp32 exp is safe), denominator from a
        ones-column appended to V (M=65 AV matmuls), division applied to
        the [64, qi] head output (commutes with the PV sum).
  attn^T [inner, tok] -> out proj produces out [tok, DIM] directly.

RoPE: rotate_every_two(q) is a fixed feat-space linear map -> done with a
single [128,128] block-diagonal matmul (Rm), then q_rot = q*cos + (Rq)*sin
elementwise on DVE; pass-dims use cos=1/sin=0 so all 64 dims are uniform.
"""

import os
import numpy as np
import ml_dtypes

import concourse.bass as bass
from concourse import bacc
import concourse.tile as tile
from concourse import mybir, library_config
from concourse.bass_utils import run_bass_kernel_spmd

BF = ml_dtypes.bfloat16
bf16 = mybir.dt.bfloat16
f32 = mybir.dt.float32

B, N, DIM, H, DH, ROT = 4, 2048, 1024, 16, 64, 32
INNER = H * DH
NQ = N // 2            # query tokens per core
NCORES = 8
P = 128
KD = DIM // P          # 8 contraction tiles over model dim
NKT = N // P           # 16 kj partition tiles
HPB = H // 2           # 8 head-pair blocks

Exp = mybir.ActivationFunctionType.Exp

_CACHE = {}


def _build_rope_consts(sin, cos):
    """cos_pad/sin_pad [128, N] for one head-pair feat block, Rm [128,128].

    Uses the provided sin/cos tables [N, ROT]; pass-dims get cos=1/sin=0 so
    RoPE applies uniformly over all 64 head dims."""
    cos_pad = np.ones((P, N), np.float32)
    sin_pad = np.zeros((P, N), np.float32)
    for half in range(2):                                # two heads per block
        r0 = half * DH
        cos_pad[r0:r0 + ROT, :] = cos.T
        sin_pad[r0:r0 + ROT, :] = sin.T

    # Rm[dp, d]: out[d] = sum_dp Rm[dp, d] * q[dp]  == rotate_every_two(q)[d]
    Rm = np.zeros((P, P), np.float32)
    for half in range(2):
        r0 = half * DH
        for i in range(0, ROT, 2):
            Rm[r0 + i + 1, r0 + i] = -1.0                # out[2i]   = -q[2i+1]
            Rm[r0 + i, r0 + i + 1] = 1.0                 # out[2i+1] =  q[2i]
    return cos_pad, sin_pad, Rm


def _build_program():
    nc = bacc.Bacc(trn_type="TRN2")

    xkv_d = nc.dram_tensor("xkv", [DIM, N], bf16, kind="ExternalInput")
    wq_d = nc.dram_tensor("wq", [DIM, INNER], bf16, kind="ExternalInput")
    wk_d = nc.dram_tensor("wk", [DIM, INNER], bf16, kind="ExternalInput")
    wv_d = nc.dram_tensor("wv", [DIM, INNER], bf16, kind="ExternalInput")
    wo_d = nc.dram_tensor("wo", [INNER, DIM], bf16, kind="ExternalInput")
    cosk_d = nc.dram_tensor("cosk", [P, N], bf16, kind="ExternalInput")
    sink_d = nc.dram_tensor("sink", [P, N], bf16, kind="ExternalInput")
    rm_d = nc.dram_tensor("rm", [P, P], bf16, kind="ExternalInput")
    out_d = nc.dram_tensor("out", [NQ, DIM], f32, kind="ExternalOutput")

    with tile.TileContext(nc) as tc:
        with (
            tc.tile_pool(name="res", bufs=1) as res,          # kernel-lifetime tiles
            tc.tile_pool(name="kstream", bufs=2) as kstream,  # per-hp q/k tiles
            tc.tile_pool(name="wstream", bufs=1) as wstream,
            tc.tile_pool(name="pt", bufs=4) as ptp,           # P^T tiles
            tc.tile_pool(name="tmp", bufs=4) as tmp,          # rope DVE temps
            tc.tile_pool(name="small", bufs=2) as small,
            tc.tile_pool(name="ostage", bufs=3) as ostage,
            tc.tile_pool(name="psA", bufs=2, space="PSUM") as psA,    # [128,512] proj/outproj/swap
            tc.tile_pool(name="psS", bufs=2, space="PSUM") as psS,    # [128,1024] scores
            tc.tile_pool(name="psV", bufs=2, space="PSUM") as psV,    # [65,512] AV
        ):
            nc.gpsimd.load_library(library_config.attn)

            # ---- resident loads (small rope consts first, K-weights last) ----
            cosk = res.tile([P, N], bf16, tag="cosk")
            sink = res.tile([P, N], bf16, tag="sink")
            rm = res.tile([P, P], bf16, tag="rm")
            for t, d in ((rm, rm_d), (cosk, cosk_d), (sink, sink_d)):
                nc.sync.dma_start(t[:], d[:])
            xkv, wq, wk = [], [], []
            for k in range(KD):
                t = res.tile([P, N], bf16, tag=f"xkv{k}", name=f"xkv{k}")
                nc.sync.dma_start(t[:], xkv_d[k * P:(k + 1) * P, :])
                xkv.append(t)
                t = res.tile([P, DIM], bf16, tag=f"wq{k}", name=f"wq{k}")
                nc.sync.dma_start(t[:], wq_d[k * P:(k + 1) * P, :])
                wq.append(t)
            for k in range(KD):
                t = res.tile([P, DIM], bf16, tag=f"wk{k}", name=f"wk{k}")
                nc.sync.dma_start(t[:], wk_d[k * P:(k + 1) * P, :])
                wk.append(t)

            attnT = []
            for k in range(KD):
                attnT.append(res.tile([P, NQ], bf16, tag=f"attnT{k}", name=f"attnT{k}"))
            vaug = []
            for mt in range(NKT):
                vt = res.tile([P, HPB, 2, 65], bf16, tag=f"vaug{mt}", name=f"vaug{mt}")
                nc.vector.memset(vt[:, :, :, 64], 1.0)
                vaug.append(vt)

            def emit_vproj(bn):
                """Project V feats [bn*512, bn*512+512) = hp blocks 4bn..4bn+3."""
                wvt = []
                for k in range(KD):
                    t = wstream.tile([P, 512], bf16, tag=f"wv{k}", name=f"wv{k}")
                    nc.sync.dma_start(t[:], wv_d[k * P:(k + 1) * P,
                                                 bn * 512:(bn + 1) * 512])
                    wvt.append(t)
                for mt in range(NKT):
                    ps = psA.tile([P, 512], f32, tag="ps")
                    for k in range(KD):
                        nc.tensor.matmul(ps[:], xkv[k][:, mt * P:(mt + 1) * P],
                                         wvt[k][:],
                                         start=(k == 0), stop=(k == KD - 1))
                    nc.vector.tensor_copy(
                        vaug[mt][:, bn * 4:(bn + 1) * 4, :, 0:64],
                        ps[:].rearrange("p (b h d) -> p b h d", b=4, h=2))

            state = {}

            def emit_proj(hp):
                """Project+rope feat block hp (heads 2hp, 2hp+1)."""
                c0 = hp * P
                # --- q^T block: [128 feats, NQ]  (q tokens = xkv cols 0:NQ) ---
                qraw = kstream.tile([P, NQ], bf16, tag="qraw")
                for n in range(NQ // 512):
                    ps = psA.tile([P, 512], f32, tag="ps")
                    for k in range(KD):
                        nc.tensor.matmul(ps[:], wq[k][:, c0:c0 + P],
                                         xkv[k][:, n * 512:(n + 1) * 512],
                                         start=(k == 0), stop=(k == KD - 1))
                    nc.vector.tensor_copy(qraw[:, n * 512:(n + 1) * 512], ps[:])
                qrot = kstream.tile([P, NQ], bf16, tag="qrot")
                for n in range(NQ // 512):
                    sl = slice(n * 512, (n + 1) * 512)
                    psw = psA.tile([P, 512], f32, tag="ps")
                    nc.tensor.matmul(psw[:], rm[:], qraw[:, sl], start=True, stop=True)
                    t1 = tmp.tile([P, 512], bf16, tag="t1")
                    nc.vector.tensor_mul(t1[:], qraw[:, sl], cosk[:, sl])
                    t2 = tmp.tile([P, 512], bf16, tag="t2")
                    nc.vector.tensor_mul(t2[:], psw[:], sink[:, sl])
                    nc.vector.tensor_add(qrot[:, sl], t1[:], t2[:])
                # --- k^T block: [128 feats, N] ---
                kraw = kstream.tile([P, N], bf16, tag="kraw")
                for n in range(N // 512):
                    ps = psA.tile([P, 512], f32, tag="ps")
                    for k in range(KD):
                        nc.tensor.matmul(ps[:], wk[k][:, c0:c0 + P],
                                         xkv[k][:, n * 512:(n + 1) * 512],
                                         start=(k == 0), stop=(k == KD - 1))
                    nc.vector.tensor_copy(kraw[:, n * 512:(n + 1) * 512], ps[:])
                krot = kstream.tile([P, N], bf16, tag="krot")
                for n in range(N // 512):
                    sl = slice(n * 512, (n + 1) * 512)
                    psw = psA.tile([P, 512], f32, tag="ps")
                    nc.tensor.matmul(psw[:], rm[:], kraw[:, sl], start=True, stop=True)
                    t1 = tmp.tile([P, 512], bf16, tag="t1")
                    nc.vector.tensor_mul(t1[:], kraw[:, sl], cosk[:, sl])
                    t2 = tmp.tile([P, 512], bf16, tag="t2")
                    nc.vector.tensor_mul(t2[:], psw[:], sink[:, sl])
                    nc.vector.tensor_add(krot[:, sl], t1[:], t2[:])
                state[hp] = (qrot, krot)

            def emit_attn_qk(hp, half):
                qrot, krot = state[hp]
                hoff = half * DH
                pts = []
                for kt in range(NKT):
                    ps = psS.tile([P, NQ], f32, tag="s")
                    for qn in range(NQ // 512):
                        nc.tensor.matmul(
                            ps[:, qn * 512:(qn + 1) * 512],
                            krot[hoff:hoff + DH, kt * P:(kt + 1) * P],
                            qrot[hoff:hoff + DH, qn * 512:(qn + 1) * 512],
                            start=True, stop=True)
                    pt = ptp.tile([P, NQ], bf16, tag="pt")
                    nc.scalar.activation(pt[:], ps[:], Exp)
                    pts.append(pt)
                return pts

            def emit_attn_av(hp, half, pts):
                hoff = half * DH
                pvs = [psV.tile([65, 512], f32, tag="av", name="av")
                       for _ in range(NQ // 512)]
                for kt in range(NKT):
                    for qn in range(NQ // 512):
                        nc.tensor.matmul(pvs[qn][:], vaug[kt][:, hp, half, :],
                                         pts[kt][:, qn * 512:(qn + 1) * 512],
                                         start=(kt == 0), stop=(kt == NKT - 1))
                for qn in range(NQ // 512):
                    sl = slice(qn * 512, (qn + 1) * 512)
                    pv = pvs[qn]
                    rec = small.tile([1, 512], f32, tag="rec")
                    nc.vector.reciprocal(rec[:], pv[64:65, :])
                    recb = small.tile([64, 512], f32, tag="recb")
                    nc.gpsimd.partition_broadcast(recb[:], rec[:])
                    nc.vector.tensor_mul(attnT[hp][hoff:hoff + DH, sl],
                                         pv[0:64, :], recb[:])

            def prefetch_wo(n):
                wot = []
                for k in range(KD):
                    t = wstream.tile([P, 512], bf16, tag=f"wo{n}_{k}",
                                     name=f"wo{n}_{k}", bufs=1)
                    nc.sync.dma_start(t[:], wo_d[k * P:(k + 1) * P,
                                                 n * 512:(n + 1) * 512])
                    wot.append(t)
                return wot

            wo_pre = {}
            emit_proj(0)
            emit_vproj(0)
            for hp in range(HPB):
                pts0 = emit_attn_qk(hp, 0)
                emit_attn_av(hp, 0, pts0)
                if hp + 1 < HPB:
                    emit_proj(hp + 1)
                pts1 = emit_attn_qk(hp, 1)
                if hp == 2:
                    emit_vproj(1)
                if hp == HPB - 2:
                    wo_pre[0] = prefetch_wo(0)
                if hp == HPB - 1:
                    wo_pre[1] = prefetch_wo(1)
                emit_attn_av(hp, 1, pts1)
                state.pop(hp)

            # ---- out projection: out[tok, DIM] = attnT.T @ Wout ----
            for n in range(DIM // 512):
                wot = wo_pre[n]
                for mt in range(NQ // P):
                    ps = psA.tile([P, 512], f32, tag="ps")
                    for k in range(KD):
                        nc.tensor.matmul(ps[:], attnT[k][:, mt * P:(mt + 1) * P],
                                         wot[k][:],
                                         start=(k == 0), stop=(k == KD - 1))
                    st = ostage.tile([P, 512], f32, tag="ost")
                    nc.scalar.copy(st[:], ps[:])
                    nc.sync.dma_start(
                        out_d[mt * P:(mt + 1) * P, n * 512:(n + 1) * 512], st[:])

    nc.compile()
    return nc


def _prep_inputs(x, sin, cos, Wqkv, Wout):
    """Host-side sharding/layout prep. Returns in_maps list for 8 cores."""
    x = np.asarray(x, np.float32)
    Wqkv = np.asarray(Wqkv, np.float32)
    Wout = np.asarray(Wout, np.float32)
    scale = DH ** -0.5
    wq = (Wqkv[:, :INNER] * scale).astype(BF)
    wk = Wqkv[:, INNER:2 * INNER].astype(BF)
    wv = Wqkv[:, 2 * INNER:].astype(BF)
    wo = Wout.astype(BF)
    cos_pad, sin_pad, Rm = _build_rope_consts(
        np.asarray(sin, np.float32), np.asarray(cos, np.float32))
    rm = Rm.astype(BF)

    in_maps = []
    for c in range(NCORES):
        b, half = divmod(c, 2)
        xT = np.ascontiguousarray(x[b].T)                          # [DIM, N]
        ck, sk = cos_pad, sin_pad
        if half == 1:        # rotate tokens so this core's queries come first
            xT = np.concatenate([xT[:, NQ:], xT[:, :NQ]], axis=1)
            ck = np.concatenate([ck[:, NQ:], ck[:, :NQ]], axis=1)
            sk = np.concatenate([sk[:, NQ:], sk[:, :NQ]], axis=1)
        in_maps.append({
            "xkv": np.ascontiguousarray(xT).astype(BF),
            "wq": wq, "wk": wk, "wv": wv, "wo": wo,
            "cosk": np.ascontiguousarray(ck).astype(BF),
            "sink": np.ascontiguousarray(sk).astype(BF),
            "rm": rm,
        })
    return in_maps


LAST_RESULTS = None


def kernel(x, sin, cos, Wqkv, Wout):
    global LAST_RESULTS
    if "nc" not in _CACHE:
        _CACHE["nc"] = _build_program()
    nc = _CACHE["nc"]
    in_maps = _prep_inputs(x, sin, cos, Wqkv, Wout)
    trace = bool(int(os.environ.get("KERNEL_TRACE", "0")))
    try:
        res = run_bass_kernel_spmd(nc, in_maps, core_ids=list(range(NCORES)),
                                   trace=trace)
    except (ImportError, ModuleNotFoundError):
        # NTFF profiling hook unavailable in this environment
        res = run_bass_kernel_spmd(nc, in_maps, core_ids=list(range(NCORES)),
                                   trace=False)
    LAST_RESULTS = res
    out = np.empty((B, N, DIM), np.float32)
    for c in range(NCORES):
        b, half = divmod(c, 2)
        out[b, half * NQ:(half + 1) * NQ, :] = res.results[c]["out"]
    return out



# revision 4
# speedup vs baseline: 1.0180x; 1.0180x over previous
"""Trainium2 Bass kernel for nn_Attention_40492951666725.

Full attention layer: qkv proj -> RoPE (interleaved pairs, rot dim 32) ->
softmax(QK^T)V -> out proj.  B=4, N=2048, DIM=1024, H=16, DH=64.

Sharding: 8 cores, core c handles batch b=c//2 and query-half c%2 (1024
query tokens, all 16 heads, full 2048-token K/V).  K/V projection is
computed redundantly by the two cores sharing a batch; no collectives.
The host rotates the token axis per core so the core's own query tokens
are always columns [0:1024] of xT (attention is permutation-invariant
over keys, so k/v/cos/sin just follow the same order).

Layouts (per core):
  xT   [DIM, 2048]  (host-transposed)   -> lhsT/rhs for projections
  q^T  [feat, 1024], k^T [feat, 2048]   feat on partitions
  S^T  [kj, qi]  (kj on partitions)     -> softmax via exp (no max-sub;
        scores are O(+-10) so fp32 exp is safe), denominator from a
        ones-column appended to V (M=65 AV matmuls), division applied to
        the [64, qi] head output (commutes with the PV sum).
  attn^T [inner, tok] -> out proj produces out [tok, DIM] directly.

RoPE: rotate_every_two(q) is a fixed feat-space linear map -> done with a
single [128,128] block-diagonal matmul (Rm), then q_rot = q*cos + (Rq)*sin
elementwise on DVE; pass-dims use cos=1/sin=0 so all 64 dims are uniform.
"""

import os
import numpy as np
import ml_dtypes

import concourse.bass as bass
from concourse import bacc
import concourse.tile as tile
from concourse import mybir, library_config
from concourse.bass_utils import run_bass_kernel_spmd

BF = ml_dtypes.bfloat16
bf16 = mybir.dt.bfloat16
f32 = mybir.dt.float32

B, N, DIM, H, DH, ROT = 4, 2048, 1024, 16, 64, 32
INNER = H * DH
NQ = N // 2            # query tokens per core
NCORES = 8
P = 128
KD = DIM // P          # 8 contraction tiles over model dim
NKT = N // P           # 16 kj partition tiles
HPB = H // 2           # 8 head-pair blocks

Exp = mybir.ActivationFunctionType.Exp

_CACHE = {}


def _build_rope_consts(sin, cos):
    """cos_pad/sin_pad [128, N] for one head-pair feat block, Rm [128,128].

    Uses the provided sin/cos tables [N, ROT]; pass-dims get cos=1/sin=0 so
    RoPE applies uniformly over all 64 head dims."""
    cos_pad = np.ones((P, N), np.float32)
    sin_pad = np.zeros((P, N), np.float32)
    for half in range(2):                                # two heads per block
        r0 = half * DH
        cos_pad[r0:r0 + ROT, :] = cos.T
        sin_pad[r0:r0 + ROT, :] = sin.T

    # Rm[dp, d]: out[d] = sum_dp Rm[dp, d] * q[dp]  == rotate_every_two(q)[d]
    Rm = np.zeros((P, P), np.float32)
    for half in range(2):
        r0 = half * DH
        for i in range(0, ROT, 2):
            Rm[r0 + i + 1, r0 + i] = -1.0                # out[2i]   = -q[2i+1]
            Rm[r0 + i, r0 + i + 1] = 1.0                 # out[2i+1] =  q[2i]
    return cos_pad, sin_pad, Rm


def _build_program():
    nc = bacc.Bacc(trn_type="TRN2")

    xkv_d = nc.dram_tensor("xkv", [DIM, N], bf16, kind="ExternalInput")
    wq_d = nc.dram_tensor("wq", [DIM, INNER], bf16, kind="ExternalInput")
    wk_d = nc.dram_tensor("wk", [DIM, INNER], bf16, kind="ExternalInput")
    wv_d = nc.dram_tensor("wv", [DIM, INNER], bf16, kind="ExternalInput")
    wo_d = nc.dram_tensor("wo", [INNER, DIM], bf16, kind="ExternalInput")
    cosk_d = nc.dram_tensor("cosk", [P, N], bf16, kind="ExternalInput")
    sink_d = nc.dram_tensor("sink", [P, N], bf16, kind="ExternalInput")
    rm_d = nc.dram_tensor("rm", [P, P], bf16, kind="ExternalInput")
    tid_d = nc.dram_tensor("tid", [P, P], bf16, kind="ExternalInput")
    out_d = nc.dram_tensor("out", [NQ, DIM], f32, kind="ExternalOutput")

    with tile.TileContext(nc) as tc:
        with (
            tc.tile_pool(name="res", bufs=1) as res,          # kernel-lifetime tiles
            tc.tile_pool(name="kstream", bufs=2) as kstream,  # per-hp q/k tiles
            tc.tile_pool(name="wstream", bufs=1) as wstream,
            tc.tile_pool(name="pt", bufs=4) as ptp,           # P^T tiles
            tc.tile_pool(name="tmp", bufs=4) as tmp,          # rope DVE temps
            tc.tile_pool(name="small", bufs=2) as small,
            tc.tile_pool(name="ostage", bufs=2) as ostage,
            tc.tile_pool(name="astage", bufs=2) as astage,
            tc.tile_pool(name="psA", bufs=2, space="PSUM") as psA,    # [128,512] proj/outproj/transp
            tc.tile_pool(name="psS", bufs=2, space="PSUM") as psS,    # [128,1024] scores
            tc.tile_pool(name="psV", bufs=1, space="PSUM") as psV,    # 2 banks, 4 AV accums each
        ):
            nc.gpsimd.load_library(library_config.attn)

            # ---- resident loads, ordered so proj(0) starts ASAP:
            # hp=0 only needs wq/wk cols 0:128, Q blocks only xkv cols 0:NQ.
            cosk = res.tile([P, N], bf16, tag="cosk")
            sink = res.tile([P, N], bf16, tag="sink")
            rm = res.tile([P, P], bf16, tag="rm")
            ident = res.tile([P, P], bf16, tag="tid", name="tid_sb")
            for t, d in ((rm, rm_d), (ident, tid_d)):
                nc.gpsimd.dma_start(t[:], d[:])
            xkv, wq, wk = [], [], []
            for k in range(KD):
                t = res.tile([P, N], bf16, tag=f"xkv{k}", name=f"xkv{k}")
                xkv.append(t)
                w = res.tile([P, DIM], bf16, tag=f"wq{k}", name=f"wq{k}")
                wq.append(w)
                nc.gpsimd.dma_start(t[:, 0:NQ], xkv_d[k * P:(k + 1) * P, 0:NQ])
                nc.gpsimd.dma_start(w[:], wq_d[k * P:(k + 1) * P, :])
                if k == 3:
                    for ct, d in ((cosk, cosk_d), (sink, sink_d)):
                        nc.gpsimd.dma_start(ct[:], d[:])
            for k in range(KD):
                t = res.tile([P, DIM], bf16, tag=f"wk{k}", name=f"wk{k}")
                wk.append(t)
                nc.gpsimd.dma_start(t[:], wk_d[k * P:(k + 1) * P, :])
            for k in range(KD):
                nc.gpsimd.dma_start(xkv[k][:, NQ:N], xkv_d[k * P:(k + 1) * P, NQ:N])

            attnT = []
            for k in range(KD):
                attnT.append(res.tile([P, NQ], bf16, tag=f"attnT{k}", name=f"attnT{k}"))
            vaug = []
            for mt in range(NKT):
                vt = res.tile([P, HPB, 2, 65], bf16, tag=f"vaug{mt}", name=f"vaug{mt}")
                nc.vector.memset(vt[:, :, :, 64], 1.0)
                vaug.append(vt)

            def vproj_chunks(bn):
                """Chunked V projection of feats [bn*512, (bn+1)*512)."""
                wvt = []

                def cdma():
                    for k in range(KD):
                        t = wstream.tile([P, 512], bf16, tag=f"wv{k}", name=f"wv{k}")
                        nc.gpsimd.dma_start(t[:], wv_d[k * P:(k + 1) * P,
                                                       bn * 512:(bn + 1) * 512])
                        wvt.append(t)

                chunks = [cdma]
                for mt in range(NKT):
                    box = {}

                    def c1(mt=mt):
                        ps = psA.tile([P, 512], f32, tag="ps", name="ps")
                        box[0] = ps
                        for k in range(4):
                            nc.tensor.matmul(ps[:], xkv[k][:, mt * P:(mt + 1) * P],
                                             wvt[k][:],
                                             start=(k == 0), stop=False)

                    def c2(mt=mt):
                        ps = box[0]
                        for k in range(4, KD):
                            nc.tensor.matmul(ps[:], xkv[k][:, mt * P:(mt + 1) * P],
                                             wvt[k][:],
                                             start=False, stop=(k == KD - 1))
                        nc.vector.tensor_copy(
                            vaug[mt][:, bn * 4:(bn + 1) * 4, :, 0:64],
                            ps[:].rearrange("p (b h d) -> p b h d", b=4, h=2))

                    chunks += [c1, c2]
                return chunks

            state = {}

            def proj_block_chunks(dst, w, c0, n):
                """Two chunks: 4+4 matmuls accumulating one 512-token block.

                The psA 'ps' tag has bufs=2 and every chunk pair is adjacent
                in the FIFO, so the accumulator survives until its second
                chunk (at most one other 'ps' alloc in between)."""
                box = {}

                def c1():
                    ps = psA.tile([P, 512], f32, tag="ps", name="ps")
                    box[0] = ps
                    for k in range(4):
                        nc.tensor.matmul(ps[:], w[k][:, c0:c0 + P],
                                         xkv[k][:, n * 512:(n + 1) * 512],
                                         start=(k == 0), stop=False)

                def c2():
                    ps = box[0]
                    for k in range(4, KD):
                        nc.tensor.matmul(ps[:], w[k][:, c0:c0 + P],
                                         xkv[k][:, n * 512:(n + 1) * 512],
                                         start=False, stop=(k == KD - 1))
                    nc.vector.tensor_copy(dst[:, n * 512:(n + 1) * 512], ps[:])

                return [c1, c2]

            def rope_chunk(dst, n):
                sl = slice(n * 512, (n + 1) * 512)

                def c3():
                    psw = psA.tile([P, 512], f32, tag="ps", name="psw")
                    nc.tensor.matmul(psw[:], rm[:], dst[:, sl], start=True, stop=True)
                    t1 = tmp.tile([P, 512], bf16, tag="t1", name="t1")
                    nc.vector.tensor_mul(t1[:], dst[:, sl], cosk[:, sl])
                    t2 = tmp.tile([P, 512], bf16, tag="t2", name="t2")
                    nc.vector.tensor_mul(t2[:], psw[:], sink[:, sl])
                    nc.vector.tensor_add(dst[:, sl], t1[:], t2[:])

                return [c3]

            def proj_chunks(hp):
                """Chunked projection+rope of feat block hp (heads 2hp, 2hp+1)."""
                c0 = hp * P
                qraw = kstream.tile([P, NQ], bf16, tag="qraw", name="qraw")
                kraw = kstream.tile([P, N], bf16, tag="kraw", name="kraw")
                state[hp] = (qraw, kraw)
                chunks = []
                for n in range(NQ // 512):
                    chunks += proj_block_chunks(qraw, wq, c0, n)
                    chunks += rope_chunk(qraw, n)
                for n in range(N // 512):
                    chunks += proj_block_chunks(kraw, wk, c0, n)
                    chunks += rope_chunk(kraw, n)
                return chunks

            PROJ_CHUNK_COSTS = [850, 900, 250] * (NQ // 512 + N // 512)

            def emit_attn_half(hp, half, stage, pop_chunk):
                """QK -> exp -> AV for one head, kt-pipelined.

                AV uses P as stationary: out[qi, 65] = sum_kj P[kj,qi] Vaug[kj,:]
                (free dim 65 vs 512, halving tensor-engine time).  The AV psum
                banks are pre-zeroed on the idle Pool engine and accumulated
                with start=False -- hardware-wise a plain += onto zeros -- to
                sidestep the one-pending-group-per-zero-region limit while
                keeping AV right behind each exp.  pop_chunk() is called once
                per kt to splice ~0.5-1us of projection work into the PE
                stream, filling the slack left by the Act-paced exp."""
                qrot, krot = state[hp]
                hoff = half * DH
                banks = [psV.tile([P, 512], f32, tag=f"bank{i}", name=f"bank{i}")
                         for i in range(2)]
                for b in banks:
                    nc.vector.memset(b[:], 0.0)
                pvs = [banks[qt // 4][:, (qt % 4) * P:(qt % 4) * P + 65]
                       for qt in range(NQ // P)]
                for kt in range(NKT):
                    ps = psS.tile([P, NQ], f32, tag="s", name="s")
                    for qn in range(NQ // 512):
                        nc.tensor.matmul(
                            ps[:, qn * 512:(qn + 1) * 512],
                            krot[hoff:hoff + DH, kt * P:(kt + 1) * P],
                            qrot[hoff:hoff + DH, qn * 512:(qn + 1) * 512],
                            start=True, stop=True)
                    pt = ptp.tile([P, NQ], bf16, tag="pt", name="pt")
                    nc.scalar.activation(pt[:], ps[:], Exp)
                    for qt in range(NQ // P):
                        nc.tensor.matmul(pvs[qt],
                                         pt[:, qt * P:(qt + 1) * P],
                                         vaug[kt][:, hp, half, :],
                                         start=False, stop=(kt == NKT - 1),
                                         skip_group_check=True)
                    pop_chunk()
                for qt in range(NQ // P):
                    pv = pvs[qt]
                    rec = small.tile([P, 1], f32, tag="rec", name="rec")
                    nc.vector.reciprocal(rec[:], pv[:, 64:65])
                    nc.vector.tensor_scalar_mul(
                        stage[qt][:, hoff:hoff + DH], pv[:, 0:64], rec[:])
                return banks

            def emit_attn_transpose(hp, stage, banks):
                """stage[qt] [tok 128, feat 128] -> attnT[hp] [feat, tok].

                Transposes land in the (just-drained) AV psum banks, viewed
                as bf16, so no extra PSUM bank or psA tag is needed."""
                for qt in range(NQ // P):
                    tr = banks[qt // 4][:, (qt % 4) * P:(qt % 4) * P + 64].bitcast(bf16)
                    nc.tensor.transpose(tr, stage[qt][:], ident[:])
                    nc.vector.tensor_copy(attnT[hp][:, qt * P:(qt + 1) * P], tr)

            def prefetch_wo(n):
                wot = []
                for k in range(KD):
                    t = wstream.tile([P, 512], bf16, tag=f"wo{n}_{k}",
                                     name=f"wo{n}_{k}", bufs=1)
                    nc.gpsimd.dma_start(t[:], wo_d[k * P:(k + 1) * P,
                                                   n * 512:(n + 1) * 512])
                    wot.append(t)
                return wot

            # ---- main loop: attention per (hp, half), with next-hp projection
            # work spliced chunk-by-chunk into the per-kt PE slack ----
            wo_pre = {}
            queue = []          # (label, est_pe_ns, closure) FIFO of deferred work

            def pop_chunk():
                budget = 520
                while queue and budget > 0:
                    _, est, fn = queue.pop(0)
                    fn()
                    budget -= est

            def drain(label):
                while any(lb == label for lb, _, _ in queue):
                    queue.pop(0)[2]()

            for fn in proj_chunks(0):
                fn()
            for fn in vproj_chunks(0):
                fn()
            for hp in range(HPB):
                drain(f"proj{hp}")          # qraw/kraw for hp must be complete
                stage = [astage.tile([P, P], bf16, tag=f"st{qt}", name=f"st{qt}")
                         for qt in range(NQ // P)]
                if hp + 1 < HPB:
                    queue.extend((f"proj{hp + 1}", est, fn) for est, fn in
                                 zip(PROJ_CHUNK_COSTS, proj_chunks(hp + 1)))
                if hp == 1:
                    vc = vproj_chunks(1)
                    queue.extend(("vproj1", 0 if i == 0 else (850 if i % 2 else 900), fn)
                                 for i, fn in enumerate(vc))
                emit_attn_half(hp, 0, stage, pop_chunk)
                if hp == HPB - 2:
                    wo_pre[0] = prefetch_wo(0)
                if hp == HPB - 1:
                    wo_pre[1] = prefetch_wo(1)
                banks = emit_attn_half(hp, 1, stage, pop_chunk)
                state.pop(hp)
                emit_attn_transpose(hp, stage, banks)
            drain("vproj1")

            # ---- out projection: out[tok, DIM] = attnT.T @ Wout ----
            for n in range(DIM // 512):
                wot = wo_pre[n]
                for mt in range(NQ // P):
                    ps = psA.tile([P, 512], f32, tag="ps")
                    for k in range(KD):
                        nc.tensor.matmul(ps[:], attnT[k][:, mt * P:(mt + 1) * P],
                                         wot[k][:],
                                         start=(k == 0), stop=(k == KD - 1))
                    st = ostage.tile([P, 512], f32, tag="ost")
                    nc.vector.tensor_copy(st[:], ps[:])
                    nc.sync.dma_start(
                        out_d[mt * P:(mt + 1) * P, n * 512:(n + 1) * 512], st[:])

    nc.compile()
    return nc


def _prep_inputs(x, sin, cos, Wqkv, Wout):
    """Host-side sharding/layout prep. Returns in_maps list for 8 cores."""
    x = np.asarray(x, np.float32)
    Wqkv = np.asarray(Wqkv, np.float32)
    Wout = np.asarray(Wout, np.float32)
    scale = DH ** -0.5
    wq = (Wqkv[:, :INNER] * scale).astype(BF)
    wk = Wqkv[:, INNER:2 * INNER].astype(BF)
    wv = Wqkv[:, 2 * INNER:].astype(BF)
    wo = Wout.astype(BF)
    cos_pad, sin_pad, Rm = _build_rope_consts(
        np.asarray(sin, np.float32), np.asarray(cos, np.float32))
    rm = Rm.astype(BF)

    in_maps = []
    for c in range(NCORES):
        b, half = divmod(c, 2)
        xT = np.ascontiguousarray(x[b].T)                          # [DIM, N]
        ck, sk = cos_pad, sin_pad
        if half == 1:        # rotate tokens so this core's queries come first
            xT = np.concatenate([xT[:, NQ:], xT[:, :NQ]], axis=1)
            ck = np.concatenate([ck[:, NQ:], ck[:, :NQ]], axis=1)
            sk = np.concatenate([sk[:, NQ:], sk[:, :NQ]], axis=1)
        in_maps.append({
            "xkv": np.ascontiguousarray(xT).astype(BF),
            "wq": wq, "wk": wk, "wv": wv, "wo": wo,
            "cosk": np.ascontiguousarray(ck).astype(BF),
            "sink": np.ascontiguousarray(sk).astype(BF),
            "rm": rm,
            "tid": np.eye(P, dtype=np.float32).astype(BF),
        })
    return in_maps


LAST_RESULTS = None


def kernel(x, sin, cos, Wqkv, Wout):
    global LAST_RESULTS
    if "nc" not in _CACHE:
        _CACHE["nc"] = _build_program()
    nc = _CACHE["nc"]
    in_maps = _prep_inputs(x, sin, cos, Wqkv, Wout)
    trace = bool(int(os.environ.get("KERNEL_TRACE", "0")))
    try:
        res = run_bass_kernel_spmd(nc, in_maps, core_ids=list(range(NCORES)),
                                   trace=trace)
    except (ImportError, ModuleNotFoundError):
        # NTFF profiling hook unavailable in this environment
        res = run_bass_kernel_spmd(nc, in_maps, core_ids=list(range(NCORES)),
                                   trace=False)
    LAST_RESULTS = res
    out = np.empty((B, N, DIM), np.float32)
    for c in range(NCORES):
        b, half = divmod(c, 2)
        out[b, half * NQ:(half + 1) * NQ, :] = res.results[c]["out"]
    return out



# revision 5
# speedup vs baseline: 1.0630x; 1.0442x over previous
"""Trainium2 Bass kernel for nn_Attention_40492951666725.

Full attention layer: qkv proj -> RoPE (interleaved pairs, rot dim 32) ->
softmax(QK^T)V -> out proj.  B=4, N=2048, DIM=1024, H=16, DH=64.

Sharding: 8 cores, core c handles batch b=c//2 and query-half c%2 (1024
query tokens, all 16 heads, full 2048-token K/V).  K/V projection is
computed redundantly by the two cores sharing a batch; no collectives.
The host rotates the token axis per core so the core's own query tokens
are always columns [0:1024] of xT (attention is permutation-invariant
over keys, so k/v/cos/sin just follow the same order).

Layouts (per core):
  xT   [DIM, 2048]  (host-transposed)   -> lhsT/rhs for projections
  q^T  [feat, 1024], k^T [feat, 2048]   feat on partitions
  S^T  [kj, qi]  (kj on partitions)     -> softmax via exp (no max-sub;
        scores are O(+-10) so fp32 exp is safe), denominator from a
        ones-column appended to V (M=65 AV matmuls), division applied to
        the [64, qi] head output (commutes with the PV sum).
  attn^T [inner, tok] -> out proj produces out [tok, DIM] directly.

RoPE: rotate_every_two(q) is a fixed feat-space linear map -> done with a
single [128,128] block-diagonal matmul (Rm), then q_rot = q*cos + (Rq)*sin
elementwise on DVE; pass-dims use cos=1/sin=0 so all 64 dims are uniform.
"""

import os
import numpy as np
import ml_dtypes

import concourse.bass as bass
from concourse.bass import _add_dep_helper
from concourse import bacc
import concourse.tile as tile
from concourse import mybir, library_config
from concourse.bass_utils import run_bass_kernel_spmd

PAIR_GROUPS = [[0, 1], [2, 3], [4, 5], [6, 7]]
i32 = mybir.dt.int32

BF = ml_dtypes.bfloat16
bf16 = mybir.dt.bfloat16
f32 = mybir.dt.float32

B, N, DIM, H, DH, ROT = 4, 2048, 1024, 16, 64, 32
INNER = H * DH
NQ = N // 2            # query tokens per core
NCORES = 8
P = 128
KD = DIM // P          # 8 contraction tiles over model dim
NKT = N // P           # 16 kj partition tiles
HPB = H // 2           # 8 head-pair blocks

Exp = mybir.ActivationFunctionType.Exp

_CACHE = {}


def _build_rope_consts(sin, cos):
    """cos_pad/sin_pad [128, N] for one head-pair feat block, Rm [128,128].

    Uses the provided sin/cos tables [N, ROT]; pass-dims get cos=1/sin=0 so
    RoPE applies uniformly over all 64 head dims."""
    cos_pad = np.ones((P, N), np.float32)
    sin_pad = np.zeros((P, N), np.float32)
    for half in range(2):                                # two heads per block
        r0 = half * DH
        cos_pad[r0:r0 + ROT, :] = cos.T
        sin_pad[r0:r0 + ROT, :] = sin.T

    # Rm[dp, d]: out[d] = sum_dp Rm[dp, d] * q[dp]  == rotate_every_two(q)[d]
    Rm = np.zeros((P, P), np.float32)
    for half in range(2):
        r0 = half * DH
        for i in range(0, ROT, 2):
            Rm[r0 + i + 1, r0 + i] = -1.0                # out[2i]   = -q[2i+1]
            Rm[r0 + i, r0 + i + 1] = 1.0                 # out[2i+1] =  q[2i]
    return cos_pad, sin_pad, Rm


def _build_program():
    nc = bacc.Bacc(trn_type="TRN2", num_devices=NCORES)

    xkv_d = nc.dram_tensor("xkv", [DIM, N], bf16, kind="ExternalInput")
    wq_d = nc.dram_tensor("wq", [DIM, INNER], bf16, kind="ExternalInput")
    wk_d = nc.dram_tensor("wk", [DIM, INNER], bf16, kind="ExternalInput")
    wv_d = nc.dram_tensor("wv", [DIM, INNER], bf16, kind="ExternalInput")
    wo_d = nc.dram_tensor("wo", [INNER, DIM], bf16, kind="ExternalInput")
    cosk_d = nc.dram_tensor("cosk", [P, N], bf16, kind="ExternalInput")
    sink_d = nc.dram_tensor("sink", [P, N], bf16, kind="ExternalInput")
    rm_d = nc.dram_tensor("rm", [P, P], bf16, kind="ExternalInput")
    tid_d = nc.dram_tensor("tid", [P, P], bf16, kind="ExternalInput")
    par_d = nc.dram_tensor("par", [1, 1], i32, kind="ExternalInput")
    out_d = nc.dram_tensor("out", [NQ, DIM], f32, kind="ExternalOutput")
    # Pair-shared HBM staging for the K/V halves exchanged between the two
    # cores of a batch (cores 2k, 2k+1 share an HBM domain under LNC1).
    ksh_d = nc.dram_tensor("ksh", [2, HPB, P, NQ], bf16, kind="Internal",
                           addr_space="Shared")
    vsh_d = nc.dram_tensor("vsh", [2, HPB, P, 520], bf16, kind="Internal",
                           addr_space="Shared")
    bar_in_d = nc.dram_tensor("bar_in", [1, 1], f32, kind="Internal")
    bar_out_d = nc.dram_tensor("bar_out", [1, 2], f32, kind="Internal")

    with tile.TileContext(nc) as tc:
        with (
            tc.tile_pool(name="res", bufs=1) as res,          # kernel-lifetime tiles
            tc.tile_pool(name="kstream", bufs=2) as kstream,  # per-hp q/k tiles
            tc.tile_pool(name="wstream", bufs=1) as wstream,
            tc.tile_pool(name="pt", bufs=4) as ptp,           # P^T tiles
            tc.tile_pool(name="tmp", bufs=4) as tmp,          # rope DVE temps
            tc.tile_pool(name="small", bufs=2) as small,
            tc.tile_pool(name="ostage", bufs=2) as ostage,
            tc.tile_pool(name="astage", bufs=2) as astage,
            tc.tile_pool(name="psA", bufs=2, space="PSUM") as psA,    # [128,512] proj/outproj/transp
            tc.tile_pool(name="psS", bufs=2, space="PSUM") as psS,    # [128,1024] scores
            tc.tile_pool(name="psV", bufs=1, space="PSUM") as psV,    # 2 banks, 4 AV accums each
        ):
            nc.gpsimd.load_library(library_config.attn)

            # ---- pair-exchange plumbing: parity register + barrier helper ----
            par_sb = res.tile([1, 1], i32, tag="par", name="par_sb")
            nc.sync.dma_start(par_sb[:], par_d[:])
            zz = res.tile([1, 1], f32, tag="zz", name="zz")
            nc.vector.memset(zz[:], 0.0)
            nc.sync.dma_start(bar_in_d[:], zz[:])
            par_reg = nc.sync.alloc_register("par_reg")
            nc.sync.reg_load(par_reg, par_sb[0:1, 0:1])
            par = nc.sync.snap(par_reg, donate=True, min_val=0, max_val=1)

            def pair_exchange(writes_fn, reads_fn):
                """SPMD pair exchange: cond-write my slot, barrier, cond-read
                the partner's slot.  writes_fn/reads_fn(slot) emit the DMAs
                for a given shared-HBM slot index."""
                w_mine = writes_fn(0, par == 0) + writes_fn(1, par == 1)
                barr = nc.gpsimd.collective_compute(
                    "AllGather", mybir.AluOpType.bypass, PAIR_GROUPS,
                    ins=[bar_in_d[0:1, 0:1]], outs=[bar_out_d[0:1, 0:2]])
                for w in w_mine:
                    _add_dep_helper(barr.ins, w.ins, sync=True,
                                    reason="pair barrier waits for my writes")
                r_mine = reads_fn(1, par == 0) + reads_fn(0, par == 1)
                for r in r_mine:
                    _add_dep_helper(r.ins, barr.ins, sync=True,
                                    reason="partner reads gated on barrier")

            # ---- resident loads, ordered so proj(0) starts ASAP:
            # hp=0 only needs wq/wk cols 0:128, Q blocks only xkv cols 0:NQ.
            cosk = res.tile([P, N], bf16, tag="cosk")
            sink = res.tile([P, N], bf16, tag="sink")
            rm = res.tile([P, P], bf16, tag="rm")
            ident = res.tile([P, P], bf16, tag="tid", name="tid_sb")
            for t, d in ((rm, rm_d), (ident, tid_d)):
                nc.sync.dma_start(t[:], d[:])
            xkv, wq, wk = [], [], []
            for k in range(KD):
                t = res.tile([P, N], bf16, tag=f"xkv{k}", name=f"xkv{k}")
                xkv.append(t)
                w = res.tile([P, DIM], bf16, tag=f"wq{k}", name=f"wq{k}")
                wq.append(w)
                nc.sync.dma_start(t[:, 0:NQ], xkv_d[k * P:(k + 1) * P, 0:NQ])
                nc.sync.dma_start(w[:], wq_d[k * P:(k + 1) * P, :])
                if k == 3:
                    for ct, d in ((cosk, cosk_d), (sink, sink_d)):
                        nc.sync.dma_start(ct[:], d[:])
            for k in range(KD):
                t = res.tile([P, DIM], bf16, tag=f"wk{k}", name=f"wk{k}")
                wk.append(t)
                nc.sync.dma_start(t[:], wk_d[k * P:(k + 1) * P, :])
            for k in range(KD):
                nc.sync.dma_start(xkv[k][:, NQ:N], xkv_d[k * P:(k + 1) * P, NQ:N])

            attnT = []
            for k in range(KD):
                attnT.append(res.tile([P, NQ], bf16, tag=f"attnT{k}", name=f"attnT{k}"))
            vaug = []
            for mt in range(NKT):
                vt = res.tile([P, HPB, 2, 65], bf16, tag=f"vaug{mt}", name=f"vaug{mt}")
                nc.vector.memset(vt[:, :, :, 64], 1.0)
                vaug.append(vt)

            def vproj_chunks(bn):
                """Chunked V projection of feats [bn*512, (bn+1)*512)."""
                wvt = []

                def cdma():
                    for k in range(KD):
                        t = wstream.tile([P, 512], bf16, tag=f"wv{k}", name=f"wv{k}")
                        nc.sync.dma_start(t[:], wv_d[k * P:(k + 1) * P,
                                                       bn * 512:(bn + 1) * 512])
                        wvt.append(t)

                # bn=0 (features for hp0-3, needed early) is projected
                # redundantly for all 16 token tiles; bn=1 (hp4-7, needed
                # ~150us in) only for the core's own 8 tiles, with the other 8
                # arriving from the pair core via shared HBM.
                nmt = NKT if bn == 0 else NKT // 2
                chunks = [cdma]
                for mt in range(nmt):
                    box = {}

                    def c1(mt=mt):
                        ps = psA.tile([P, 512], f32, tag="ps", name="ps")
                        box[0] = ps
                        for k in range(4):
                            nc.tensor.matmul(ps[:], xkv[k][:, mt * P:(mt + 1) * P],
                                             wvt[k][:],
                                             start=(k == 0), stop=False)

                    def c2(mt=mt):
                        ps = box[0]
                        for k in range(4, KD):
                            nc.tensor.matmul(ps[:], xkv[k][:, mt * P:(mt + 1) * P],
                                             wvt[k][:],
                                             start=False, stop=(k == KD - 1))
                        nc.vector.tensor_copy(
                            vaug[mt][:, bn * 4:(bn + 1) * 4, :, 0:64],
                            ps[:].rearrange("p (b h d) -> p b h d", b=4, h=2))

                    chunks += [c1, c2]
                if bn == 1:
                    def cx():
                        pair_exchange(
                            lambda slot, cond: [
                                nc.sync.dma_start(vsh_d[slot, mt],
                                                    vaug[mt][:, 4:8, :, :],
                                                    cond=cond)
                                for mt in range(NKT // 2)],
                            lambda slot, cond: [
                                nc.sync.dma_start(vaug[mt + NKT // 2][:, 4:8, :, :],
                                                    vsh_d[slot, mt],
                                                    cond=cond)
                                for mt in range(NKT // 2)])
                    chunks.append(cx)
                return chunks

            state = {}

            def proj_block_chunks(dst, w, c0, n):
                """Two chunks: 4+4 matmuls accumulating one 512-token block.

                The psA 'ps' tag has bufs=2 and every chunk pair is adjacent
                in the FIFO, so the accumulator survives until its second
                chunk (at most one other 'ps' alloc in between)."""
                box = {}

                def c1():
                    ps = psA.tile([P, 512], f32, tag="ps", name="ps")
                    box[0] = ps
                    for k in range(4):
                        nc.tensor.matmul(ps[:], w[k][:, c0:c0 + P],
                                         xkv[k][:, n * 512:(n + 1) * 512],
                                         start=(k == 0), stop=False)

                def c2():
                    ps = box[0]
                    for k in range(4, KD):
                        nc.tensor.matmul(ps[:], w[k][:, c0:c0 + P],
                                         xkv[k][:, n * 512:(n + 1) * 512],
                                         start=False, stop=(k == KD - 1))
                    nc.vector.tensor_copy(dst[:, n * 512:(n + 1) * 512], ps[:])

                return [c1, c2]

            def rope_chunk(dst, n):
                sl = slice(n * 512, (n + 1) * 512)

                def c3():
                    psw = psA.tile([P, 512], f32, tag="ps", name="psw")
                    nc.tensor.matmul(psw[:], rm[:], dst[:, sl], start=True, stop=True)
                    t1 = tmp.tile([P, 512], bf16, tag="t1", name="t1")
                    nc.vector.tensor_mul(t1[:], dst[:, sl], cosk[:, sl])
                    t2 = tmp.tile([P, 512], bf16, tag="t2", name="t2")
                    nc.vector.tensor_mul(t2[:], psw[:], sink[:, sl])
                    nc.vector.tensor_add(dst[:, sl], t1[:], t2[:])

                return [c3]

            def proj_chunks(hp):
                """Chunked projection+rope of feat block hp (heads 2hp, 2hp+1).

                For hp >= 1 only the core's own token half of K is projected
                and rope'd; the other half arrives rope'd from the pair core
                via shared HBM (hp0 stays redundant: its kt8-15 are needed
                ~17us in, before a 15us barrier could resolve)."""
                c0 = hp * P
                qraw = kstream.tile([P, NQ], bf16, tag="qraw", name="qraw")
                kraw = kstream.tile([P, N], bf16, tag="kraw", name="kraw")
                state[hp] = (qraw, kraw)
                chunks = []
                for n in range(NQ // 512):
                    chunks += proj_block_chunks(qraw, wq, c0, n)
                    chunks += rope_chunk(qraw, n)
                khalf = N if hp == 0 else NQ
                for n in range(khalf // 512):
                    chunks += proj_block_chunks(kraw, wk, c0, n)
                    chunks += rope_chunk(kraw, n)
                if hp > 0:
                    def cx():
                        pair_exchange(
                            lambda slot, cond: [nc.sync.dma_start(
                                ksh_d[slot, hp], kraw[:, 0:NQ], cond=cond)],
                            lambda slot, cond: [nc.sync.dma_start(
                                kraw[:, NQ:N], ksh_d[slot, hp], cond=cond)])
                    chunks.append(cx)
                return chunks

            def proj_chunk_costs(hp):
                qk_blocks = NQ // 512 + (N if hp == 0 else NQ) // 512
                costs = [850, 900, 250] * qk_blocks
                if hp > 0:
                    costs.append(0)
                return costs

            # ---- attention as one flat (hp, half, kt) step stream.  The QK+exp
            # side runs LOOKAHEAD steps ahead of the AV side, crossing unit
            # boundaries, so the Act engine's exp stream never drains while
            # epilogues/transposes/memsets run between units.  AV uses P as
            # stationary (free dim 65 vs 512, halving tensor-engine time); its
            # psum banks are pre-zeroed on DVE and accumulated with
            # start=False -- hardware-wise a plain += onto zeros -- to
            # sidestep the one-pending-group-per-zero-region limit. ----
            STEPS = [(hp, half, kt) for hp in range(HPB) for half in (0, 1)
                     for kt in range(NKT)]
            pts = {}
            stages = {}
            avctx = {}

            def emit_qk_step(idx, drain):
                if idx >= len(STEPS):
                    return
                hp, half, kt = STEPS[idx]
                if half == 0 and kt == 0:
                    drain(f"proj{hp}")      # qraw/kraw for hp must be complete
                qrot, krot = state[hp]
                hoff = half * DH
                ps = psS.tile([P, NQ], f32, tag="s", name="s")
                for qn in range(NQ // 512):
                    nc.tensor.matmul(
                        ps[:, qn * 512:(qn + 1) * 512],
                        krot[hoff:hoff + DH, kt * P:(kt + 1) * P],
                        qrot[hoff:hoff + DH, qn * 512:(qn + 1) * 512],
                        start=True, stop=True)
                pt = ptp.tile([P, NQ], bf16, tag="pt", name="pt")
                nc.scalar.activation(pt[:], ps[:], Exp)
                pts[idx] = pt

            def emit_av_step(idx):
                hp, half, kt = STEPS[idx]
                hoff = half * DH
                if kt == 0:
                    if half == 0:
                        stages[hp] = [astage.tile([P, P], bf16, tag=f"st{qt}",
                                                  name=f"st{qt}")
                                      for qt in range(NQ // P)]
                    banks = [psV.tile([P, 512], f32, tag=f"bank{i}",
                                      name=f"bank{i}") for i in range(2)]
                    for b in banks:
                        nc.vector.memset(b[:], 0.0)
                    avctx[(hp, half)] = banks
                banks = avctx[(hp, half)]
                pvs = [banks[qt // 4][:, (qt % 4) * P:(qt % 4) * P + 65]
                       for qt in range(NQ // P)]
                pt = pts.pop(idx)
                for qt in range(NQ // P):
                    nc.tensor.matmul(pvs[qt],
                                     pt[:, qt * P:(qt + 1) * P],
                                     vaug[kt][:, hp, half, :],
                                     start=False, stop=(kt == NKT - 1),
                                     skip_group_check=True)
                if kt == NKT - 1:
                    stage = stages[hp]
                    for qt in range(NQ // P):
                        pv = pvs[qt]
                        rec = small.tile([P, 1], f32, tag="rec", name="rec")
                        nc.vector.reciprocal(rec[:], pv[:, 64:65])
                        nc.vector.tensor_scalar_mul(
                            stage[qt][:, hoff:hoff + DH], pv[:, 0:64], rec[:])
                    if half == 1:
                        emit_attn_transpose(hp, stages.pop(hp), banks)
                        avctx.pop((hp, 0))
                        avctx.pop((hp, 1))
                        state.pop(hp)

            def emit_attn_transpose(hp, stage, banks):
                """stage[qt] [tok 128, feat 128] -> attnT[hp] [feat, tok].

                Transposes land in the (just-drained) AV psum banks, viewed
                as bf16, so no extra PSUM bank or psA tag is needed."""
                for qt in range(NQ // P):
                    tr = banks[qt // 4][:, (qt % 4) * P:(qt % 4) * P + 64].bitcast(bf16)
                    nc.tensor.transpose(tr, stage[qt][:], ident[:])
                    nc.vector.tensor_copy(attnT[hp][:, qt * P:(qt + 1) * P], tr)

            def prefetch_wo(n):
                wot = []
                for k in range(KD):
                    t = wstream.tile([P, 512], bf16, tag=f"wo{n}_{k}",
                                     name=f"wo{n}_{k}", bufs=1)
                    nc.sync.dma_start(t[:], wo_d[k * P:(k + 1) * P,
                                                   n * 512:(n + 1) * 512])
                    wot.append(t)
                return wot

            # ---- main loop: attention per (hp, half), with next-hp projection
            # work spliced chunk-by-chunk into the per-kt PE slack ----
            wo_pre = {}
            queue = []          # (label, est_pe_ns, closure) FIFO of deferred work

            def pop_chunk():
                budget = 520
                while queue and budget > 0:
                    _, est, fn = queue.pop(0)
                    fn()
                    budget -= est

            def drain(label):
                # selective: emit only chunks with this label, preserving
                # their relative order (c1/c2 psA pairs stay adjacent)
                rest = []
                for lb, est, fn in queue:
                    if lb == label:
                        fn()
                    else:
                        rest.append((lb, est, fn))
                queue[:] = rest

            for fn in proj_chunks(0):
                fn()
            for fn in vproj_chunks(0):
                fn()
            LOOKAHEAD = 2
            for i in range(LOOKAHEAD):
                emit_qk_step(i, drain)
            for i, (hp, half, kt) in enumerate(STEPS):
                if half == 0 and kt == 0:
                    if hp + 1 < HPB:
                        queue.extend((f"proj{hp + 1}", est, fn) for est, fn in
                                     zip(proj_chunk_costs(hp + 1),
                                         proj_chunks(hp + 1)))
                    if hp == 1:
                        vc = vproj_chunks(1)
                        queue.extend(
                            ("vproj1", 0 if j == 0 else (850 if j % 2 else 900), fn)
                            for j, fn in enumerate(vc))
                    if hp == HPB - 2:
                        wo_pre[0] = prefetch_wo(0)
                    if hp == HPB - 1:
                        wo_pre[1] = prefetch_wo(1)
                emit_qk_step(i + LOOKAHEAD, drain)
                emit_av_step(i)
                pop_chunk()
            drain("vproj1")

            # ---- out projection: out[tok, DIM] = attnT.T @ Wout ----
            for n in range(DIM // 512):
                wot = wo_pre[n]
                for mt in range(NQ // P):
                    ps = psA.tile([P, 512], f32, tag="ps")
                    for k in range(KD):
                        nc.tensor.matmul(ps[:], attnT[k][:, mt * P:(mt + 1) * P],
                                         wot[k][:],
                                         start=(k == 0), stop=(k == KD - 1))
                    st = ostage.tile([P, 512], f32, tag="ost")
                    nc.vector.tensor_copy(st[:], ps[:])
                    nc.sync.dma_start(
                        out_d[mt * P:(mt + 1) * P, n * 512:(n + 1) * 512], st[:])

    nc.compile()
    return nc


def _prep_inputs(x, sin, cos, Wqkv, Wout):
    """Host-side sharding/layout prep. Returns in_maps list for 8 cores."""
    x = np.asarray(x, np.float32)
    Wqkv = np.asarray(Wqkv, np.float32)
    Wout = np.asarray(Wout, np.float32)
    scale = DH ** -0.5
    wq = (Wqkv[:, :INNER] * scale).astype(BF)
    wk = Wqkv[:, INNER:2 * INNER].astype(BF)
    wv = Wqkv[:, 2 * INNER:].astype(BF)
    wo = Wout.astype(BF)
    cos_pad, sin_pad, Rm = _build_rope_consts(
        np.asarray(sin, np.float32), np.asarray(cos, np.float32))
    rm = Rm.astype(BF)

    in_maps = []
    for c in range(NCORES):
        b, half = divmod(c, 2)
        xT = np.ascontiguousarray(x[b].T)                          # [DIM, N]
        ck, sk = cos_pad, sin_pad
        if half == 1:        # rotate tokens so this core's queries come first
            xT = np.concatenate([xT[:, NQ:], xT[:, :NQ]], axis=1)
            ck = np.concatenate([ck[:, NQ:], ck[:, :NQ]], axis=1)
            sk = np.concatenate([sk[:, NQ:], sk[:, :NQ]], axis=1)
        in_maps.append({
            "xkv": np.ascontiguousarray(xT).astype(BF),
            "wq": wq, "wk": wk, "wv": wv, "wo": wo,
            "cosk": np.ascontiguousarray(ck).astype(BF),
            "sink": np.ascontiguousarray(sk).astype(BF),
            "rm": rm,
            "tid": np.eye(P, dtype=np.float32).astype(BF),
            "par": np.array([[half]], dtype=np.int32),
        })
    return in_maps


LAST_RESULTS = None


def kernel(x, sin, cos, Wqkv, Wout):
    global LAST_RESULTS
    if "nc" not in _CACHE:
        _CACHE["nc"] = _build_program()
    nc = _CACHE["nc"]
    in_maps = _prep_inputs(x, sin, cos, Wqkv, Wout)
    trace = bool(int(os.environ.get("KERNEL_TRACE", "0")))
    try:
        res = run_bass_kernel_spmd(nc, in_maps, core_ids=list(range(NCORES)),
                                   trace=trace)
    except (ImportError, ModuleNotFoundError):
        # NTFF profiling hook unavailable in this environment
        res = run_bass_kernel_spmd(nc, in_maps, core_ids=list(range(NCORES)),
                                   trace=False)
    LAST_RESULTS = res
    out = np.empty((B, N, DIM), np.float32)
    for c in range(NCORES):
        b, half = divmod(c, 2)
        out[b, half * NQ:(half + 1) * NQ, :] = res.results[c]["out"]
    return out



# revision 6
# speedup vs baseline: 1.1343x; 1.0671x over previous
"""Trainium2 Bass kernel for nn_Attention_40492951666725.

Full attention layer: qkv proj -> RoPE (interleaved pairs, rot dim 32) ->
softmax(QK^T)V -> out proj.  B=4, N=2048, DIM=1024, H=16, DH=64.

Sharding: 8 cores, core c handles batch b=c//2 and query-half c%2 (1024
query tokens, all 16 heads, full 2048-token K/V).  K/V projection is
computed redundantly by the two cores sharing a batch; no collectives.
The host rotates the token axis per core so the core's own query tokens
are always columns [0:1024] of xT (attention is permutation-invariant
over keys, so k/v/cos/sin just follow the same order).

Layouts (per core):
  xT   [DIM, 2048]  (host-transposed)   -> lhsT/rhs for projections
  q^T  [feat, 1024], k^T [feat, 2048]   feat on partitions
  S^T  [kj, qi]  (kj on partitions)     -> softmax via exp (no max-sub;
        scores are O(+-10) so fp32 exp is safe), denominator from a
        ones-column appended to V (M=65 AV matmuls), division applied to
        the [64, qi] head output (commutes with the PV sum).
  attn^T [inner, tok] -> out proj produces out [tok, DIM] directly.

RoPE: rotate_every_two(q) is a fixed feat-space linear map -> done with a
single [128,128] block-diagonal matmul (Rm), then q_rot = q*cos + (Rq)*sin
elementwise on DVE; pass-dims use cos=1/sin=0 so all 64 dims are uniform.
"""

import os
import numpy as np
import ml_dtypes

import concourse.bass as bass
from concourse.bass import _add_dep_helper
from concourse import bacc
import concourse.tile as tile
from concourse import mybir, library_config
from concourse.bass_utils import run_bass_kernel_spmd

PAIR_GROUPS = [[0, 1], [2, 3], [4, 5], [6, 7]]
i32 = mybir.dt.int32

BF = ml_dtypes.bfloat16
bf16 = mybir.dt.bfloat16
f32 = mybir.dt.float32

B, N, DIM, H, DH, ROT = 4, 2048, 1024, 16, 64, 32
INNER = H * DH
NQ = N // 2            # query tokens per core
NCORES = 8
P = 128
KD = DIM // P          # 8 contraction tiles over model dim
NKT = N // P           # 16 kj partition tiles
HPB = H // 2           # 8 head-pair blocks

Exp = mybir.ActivationFunctionType.Exp

_CACHE = {}


def _build_rope_consts(sin, cos):
    """cos_pad/sin_pad [128, N] for one head-pair feat block, Rm [128,128].

    Uses the provided sin/cos tables [N, ROT]; pass-dims get cos=1/sin=0 so
    RoPE applies uniformly over all 64 head dims."""
    cos_pad = np.ones((P, N), np.float32)
    sin_pad = np.zeros((P, N), np.float32)
    for half in range(2):                                # two heads per block
        r0 = half * DH
        cos_pad[r0:r0 + ROT, :] = cos.T
        sin_pad[r0:r0 + ROT, :] = sin.T

    # Rm[dp, d]: out[d] = sum_dp Rm[dp, d] * q[dp]  == rotate_every_two(q)[d]
    Rm = np.zeros((P, P), np.float32)
    for half in range(2):
        r0 = half * DH
        for i in range(0, ROT, 2):
            Rm[r0 + i + 1, r0 + i] = -1.0                # out[2i]   = -q[2i+1]
            Rm[r0 + i, r0 + i + 1] = 1.0                 # out[2i+1] =  q[2i]
    return cos_pad, sin_pad, Rm


def _build_program():
    nc = bacc.Bacc(trn_type="TRN2", num_devices=NCORES)

    xkv_d = nc.dram_tensor("xkv", [DIM, N], bf16, kind="ExternalInput")
    wq_d = nc.dram_tensor("wq", [DIM, INNER], bf16, kind="ExternalInput")
    wk_d = nc.dram_tensor("wk", [DIM, INNER], bf16, kind="ExternalInput")
    wv_d = nc.dram_tensor("wv", [DIM, INNER], bf16, kind="ExternalInput")
    wo_d = nc.dram_tensor("wo", [INNER, DIM], bf16, kind="ExternalInput")
    cosk_d = nc.dram_tensor("cosk", [P, N], bf16, kind="ExternalInput")
    sink_d = nc.dram_tensor("sink", [P, N], bf16, kind="ExternalInput")
    rm_d = nc.dram_tensor("rm", [P, P], bf16, kind="ExternalInput")
    tid_d = nc.dram_tensor("tid", [P, P], bf16, kind="ExternalInput")
    par_d = nc.dram_tensor("par", [1, 1], i32, kind="ExternalInput")
    out_d = nc.dram_tensor("out", [NQ, DIM], f32, kind="ExternalOutput")
    # Pair-shared HBM staging for the K/V halves exchanged between the two
    # cores of a batch (cores 2k, 2k+1 share an HBM domain under LNC1).
    ksh_d = nc.dram_tensor("ksh", [2, HPB, P, NQ], bf16, kind="Internal",
                           addr_space="Shared")
    vsh_d = nc.dram_tensor("vsh", [2, HPB, P, 520], bf16, kind="Internal",
                           addr_space="Shared")
    bar_in_d = nc.dram_tensor("bar_in", [1, 1], f32, kind="Internal")
    bar_out_d = nc.dram_tensor("bar_out", [1, 2], f32, kind="Internal")

    with tile.TileContext(nc) as tc:
        with (
            tc.tile_pool(name="res", bufs=1) as res,          # kernel-lifetime tiles
            tc.tile_pool(name="kstream", bufs=2) as kstream,  # per-hp q/k tiles
            tc.tile_pool(name="wstream", bufs=1) as wstream,
            tc.tile_pool(name="pt", bufs=7) as ptp,           # P^T tiles
            tc.tile_pool(name="tmp", bufs=4) as tmp,          # rope DVE temps
            tc.tile_pool(name="small", bufs=2) as small,
            tc.tile_pool(name="ostage", bufs=2) as ostage,
            tc.tile_pool(name="astage", bufs=2) as astage,
            tc.tile_pool(name="psA", bufs=2, space="PSUM") as psA,    # [128,512] proj/outproj/transp
            tc.tile_pool(name="psS", bufs=2, space="PSUM") as psS,    # [128,1024] scores
            tc.tile_pool(name="psV", bufs=1, space="PSUM") as psV,    # 2 banks, 4 AV accums each
        ):
            nc.gpsimd.load_library(library_config.attn)

            # ---- pair-exchange plumbing: parity register + barrier helper ----
            par_sb = res.tile([1, 1], i32, tag="par", name="par_sb")
            nc.sync.dma_start(par_sb[:], par_d[:])
            zz = res.tile([1, 1], f32, tag="zz", name="zz")
            nc.vector.memset(zz[:], 0.0)
            nc.sync.dma_start(bar_in_d[:], zz[:])
            par_reg = nc.sync.alloc_register("par_reg")
            nc.sync.reg_load(par_reg, par_sb[0:1, 0:1])
            par = nc.sync.snap(par_reg, donate=True, min_val=0, max_val=1)

            def pair_exchange(writes_fn, reads_fn):
                """SPMD pair exchange: cond-write my slot, barrier, cond-read
                the partner's slot.  writes_fn/reads_fn(slot) emit the DMAs
                for a given shared-HBM slot index."""
                w_mine = writes_fn(0, par == 0) + writes_fn(1, par == 1)
                barr = nc.gpsimd.collective_compute(
                    "AllGather", mybir.AluOpType.bypass, PAIR_GROUPS,
                    ins=[bar_in_d[0:1, 0:1]], outs=[bar_out_d[0:1, 0:2]])
                for w in w_mine:
                    _add_dep_helper(barr.ins, w.ins, sync=True,
                                    reason="pair barrier waits for my writes")
                r_mine = reads_fn(1, par == 0) + reads_fn(0, par == 1)
                for r in r_mine:
                    _add_dep_helper(r.ins, barr.ins, sync=True,
                                    reason="partner reads gated on barrier")

            # ---- resident loads, ordered so proj(0) starts ASAP:
            # hp=0 only needs wq/wk cols 0:128, Q blocks only xkv cols 0:NQ.
            cosk = res.tile([P, N], bf16, tag="cosk")
            sink = res.tile([P, N], bf16, tag="sink")
            rm = res.tile([P, P], bf16, tag="rm")
            ident = res.tile([P, P], bf16, tag="tid", name="tid_sb")
            for t, d in ((rm, rm_d), (ident, tid_d)):
                nc.sync.dma_start(t[:], d[:])
            xkv, wq, wk = [], [], []
            for k in range(KD):
                t = res.tile([P, N], bf16, tag=f"xkv{k}", name=f"xkv{k}")
                xkv.append(t)
                w = res.tile([P, DIM], bf16, tag=f"wq{k}", name=f"wq{k}")
                wq.append(w)
                nc.sync.dma_start(t[:, 0:NQ], xkv_d[k * P:(k + 1) * P, 0:NQ])
                nc.sync.dma_start(w[:], wq_d[k * P:(k + 1) * P, :])
                if k == 3:
                    for ct, d in ((cosk, cosk_d), (sink, sink_d)):
                        nc.sync.dma_start(ct[:], d[:])
            for k in range(KD):
                t = res.tile([P, DIM], bf16, tag=f"wk{k}", name=f"wk{k}")
                wk.append(t)
                nc.sync.dma_start(t[:], wk_d[k * P:(k + 1) * P, :])
            for k in range(KD):
                nc.sync.dma_start(xkv[k][:, NQ:N], xkv_d[k * P:(k + 1) * P, NQ:N])

            attnT = []
            for k in range(KD):
                attnT.append(res.tile([P, NQ], bf16, tag=f"attnT{k}", name=f"attnT{k}"))
            vaug = []
            for mt in range(NKT):
                vt = res.tile([P, HPB, 2, 65], bf16, tag=f"vaug{mt}", name=f"vaug{mt}")
                nc.vector.memset(vt[:, :, :, 64], 1.0)
                vaug.append(vt)

            def vproj_chunks(bn):
                """Chunked V projection of feats [bn*512, (bn+1)*512)."""
                wvt = []

                def cdma():
                    for k in range(KD):
                        t = wstream.tile([P, 512], bf16, tag=f"wv{k}", name=f"wv{k}")
                        nc.sync.dma_start(t[:], wv_d[k * P:(k + 1) * P,
                                                       bn * 512:(bn + 1) * 512])
                        wvt.append(t)

                # bn=0 (features for hp0-3, needed early) is projected
                # redundantly for all 16 token tiles; bn=1 (hp4-7, needed
                # ~150us in) only for the core's own 8 tiles, with the other 8
                # arriving from the pair core via shared HBM.
                nmt = NKT if bn == 0 else NKT // 2
                chunks = [cdma]
                for mt in range(nmt):
                    box = {}

                    def c1(mt=mt):
                        ps = psA.tile([P, 512], f32, tag="ps", name="ps")
                        box[0] = ps
                        for k in range(4):
                            nc.tensor.matmul(ps[:], xkv[k][:, mt * P:(mt + 1) * P],
                                             wvt[k][:],
                                             start=(k == 0), stop=False)

                    def c2(mt=mt):
                        ps = box[0]
                        for k in range(4, KD):
                            nc.tensor.matmul(ps[:], xkv[k][:, mt * P:(mt + 1) * P],
                                             wvt[k][:],
                                             start=False, stop=(k == KD - 1))
                        nc.vector.tensor_copy(
                            vaug[mt][:, bn * 4:(bn + 1) * 4, :, 0:64],
                            ps[:].rearrange("p (b h d) -> p b h d", b=4, h=2))

                    chunks += [c1, c2]
                if bn == 1:
                    def cx():
                        pair_exchange(
                            lambda slot, cond: [
                                nc.sync.dma_start(vsh_d[slot, mt],
                                                    vaug[mt][:, 4:8, :, :],
                                                    cond=cond)
                                for mt in range(NKT // 2)],
                            lambda slot, cond: [
                                nc.sync.dma_start(vaug[mt + NKT // 2][:, 4:8, :, :],
                                                    vsh_d[slot, mt],
                                                    cond=cond)
                                for mt in range(NKT // 2)])
                    chunks.append(cx)
                return chunks

            state = {}

            def proj_block_chunks(dst, w, c0, n):
                """Two chunks: 4+4 matmuls accumulating one 512-token block.

                The psA 'ps' tag has bufs=2 and every chunk pair is adjacent
                in the FIFO, so the accumulator survives until its second
                chunk (at most one other 'ps' alloc in between)."""
                box = {}

                def c1():
                    ps = psA.tile([P, 512], f32, tag="ps", name="ps")
                    box[0] = ps
                    for k in range(4):
                        nc.tensor.matmul(ps[:], w[k][:, c0:c0 + P],
                                         xkv[k][:, n * 512:(n + 1) * 512],
                                         start=(k == 0), stop=False)

                def c2():
                    ps = box[0]
                    for k in range(4, KD):
                        nc.tensor.matmul(ps[:], w[k][:, c0:c0 + P],
                                         xkv[k][:, n * 512:(n + 1) * 512],
                                         start=False, stop=(k == KD - 1))
                    nc.vector.tensor_copy(dst[:, n * 512:(n + 1) * 512], ps[:])

                return [c1, c2]

            def rope_chunk(dst, n):
                sl = slice(n * 512, (n + 1) * 512)

                def c3():
                    psw = psA.tile([P, 512], f32, tag="ps", name="psw")
                    nc.tensor.matmul(psw[:], rm[:], dst[:, sl], start=True, stop=True)
                    t1 = tmp.tile([P, 512], bf16, tag="t1", name="t1")
                    nc.vector.tensor_mul(t1[:], dst[:, sl], cosk[:, sl])
                    t2 = tmp.tile([P, 512], bf16, tag="t2", name="t2")
                    nc.vector.tensor_mul(t2[:], psw[:], sink[:, sl])
                    nc.vector.tensor_add(dst[:, sl], t1[:], t2[:])

                return [c3]

            def proj_chunks(hp):
                """Chunked projection+rope of feat block hp (heads 2hp, 2hp+1).

                For hp >= 1 only the core's own token half of K is projected
                and rope'd; the other half arrives rope'd from the pair core
                via shared HBM (hp0 stays redundant: its kt8-15 are needed
                ~17us in, before a 15us barrier could resolve)."""
                c0 = hp * P
                qraw = kstream.tile([P, NQ], bf16, tag="qraw", name="qraw")
                kraw = kstream.tile([P, N], bf16, tag="kraw", name="kraw")
                state[hp] = (qraw, kraw)
                chunks = []
                for n in range(NQ // 512):
                    chunks += proj_block_chunks(qraw, wq, c0, n)
                    chunks += rope_chunk(qraw, n)
                khalf = N if hp == 0 else NQ
                for n in range(khalf // 512):
                    chunks += proj_block_chunks(kraw, wk, c0, n)
                    chunks += rope_chunk(kraw, n)
                if hp > 0:
                    def cx():
                        pair_exchange(
                            lambda slot, cond: [nc.sync.dma_start(
                                ksh_d[slot, hp], kraw[:, 0:NQ], cond=cond)],
                            lambda slot, cond: [nc.sync.dma_start(
                                kraw[:, NQ:N], ksh_d[slot, hp], cond=cond)])
                    chunks.append(cx)
                return chunks

            def proj_chunk_costs(hp):
                qk_blocks = NQ // 512 + (N if hp == 0 else NQ) // 512
                costs = [850, 900, 250] * qk_blocks
                if hp > 0:
                    costs.append(0)
                return costs

            # ---- attention as one flat (hp, half, kt) step stream.  The QK+exp
            # side runs LOOKAHEAD steps ahead of the AV side, crossing unit
            # boundaries, so the Act engine's exp stream never drains while
            # epilogues/transposes/memsets run between units.  AV uses P as
            # stationary (free dim 65 vs 512, halving tensor-engine time); its
            # psum banks are pre-zeroed on DVE and accumulated with
            # start=False -- hardware-wise a plain += onto zeros -- to
            # sidestep the one-pending-group-per-zero-region limit. ----
            STEPS = [(hp, half, kt) for hp in range(HPB) for half in (0, 1)
                     for kt in range(NKT)]
            pts = {}
            stages = {}
            avctx = {}

            def emit_qk_step(idx, drain):
                if idx >= len(STEPS):
                    return
                hp, half, kt = STEPS[idx]
                if half == 0 and kt == 0:
                    drain(f"proj{hp}")      # qraw/kraw for hp must be complete
                qrot, krot = state[hp]
                hoff = half * DH
                ps = psS.tile([P, NQ], f32, tag="s", name="s")
                for qn in range(NQ // 512):
                    nc.tensor.matmul(
                        ps[:, qn * 512:(qn + 1) * 512],
                        krot[hoff:hoff + DH, kt * P:(kt + 1) * P],
                        qrot[hoff:hoff + DH, qn * 512:(qn + 1) * 512],
                        start=True, stop=True)
                pt = ptp.tile([P, NQ], bf16, tag="pt", name="pt")
                nc.scalar.activation(pt[:], ps[:], Exp)
                pts[idx] = pt

            def emit_av_step(idx):
                hp, half, kt = STEPS[idx]
                hoff = half * DH
                if kt == 0:
                    if half == 0:
                        stages[hp] = [astage.tile([P, P], bf16, tag=f"st{qt}",
                                                  name=f"st{qt}")
                                      for qt in range(NQ // P)]
                    banks = [psV.tile([P, 512], f32, tag=f"bank{i}",
                                      name=f"bank{i}") for i in range(2)]
                    for b in banks:
                        nc.vector.memset(b[:], 0.0)
                    avctx[(hp, half)] = banks
                banks = avctx[(hp, half)]
                pvs = [banks[qt // 4][:, (qt % 4) * P:(qt % 4) * P + 65]
                       for qt in range(NQ // P)]
                pt = pts.pop(idx)
                for qt in range(NQ // P):
                    nc.tensor.matmul(pvs[qt],
                                     pt[:, qt * P:(qt + 1) * P],
                                     vaug[kt][:, hp, half, :],
                                     start=False, stop=(kt == NKT - 1),
                                     skip_group_check=True)
                if kt == NKT - 1:
                    stage = stages[hp]
                    for qt in range(NQ // P):
                        pv = pvs[qt]
                        rec = small.tile([P, 1], f32, tag="rec", name="rec")
                        nc.vector.reciprocal(rec[:], pv[:, 64:65])
                        nc.vector.tensor_scalar_mul(
                            stage[qt][:, hoff:hoff + DH], pv[:, 0:64], rec[:])
                    if half == 1:
                        emit_attn_transpose(hp, stages.pop(hp), banks)
                        avctx.pop((hp, 0))
                        avctx.pop((hp, 1))
                        state.pop(hp)

            def emit_attn_transpose(hp, stage, banks):
                """stage[qt] [tok 128, feat 128] -> attnT[hp] [feat, tok].

                Transposes land in the (just-drained) AV psum banks, viewed
                as bf16, so no extra PSUM bank or psA tag is needed."""
                for qt in range(NQ // P):
                    tr = banks[qt // 4][:, (qt % 4) * P:(qt % 4) * P + 64].bitcast(bf16)
                    nc.tensor.transpose(tr, stage[qt][:], ident[:])
                    nc.vector.tensor_copy(attnT[hp][:, qt * P:(qt + 1) * P], tr)

            def prefetch_wo(n):
                wot = []
                for k in range(KD):
                    t = wstream.tile([P, 512], bf16, tag=f"wo{n}_{k}",
                                     name=f"wo{n}_{k}", bufs=1)
                    nc.sync.dma_start(t[:], wo_d[k * P:(k + 1) * P,
                                                   n * 512:(n + 1) * 512])
                    wot.append(t)
                return wot

            # ---- main loop: attention per (hp, half), with next-hp projection
            # work spliced chunk-by-chunk into the per-kt PE slack ----
            wo_pre = {}
            queue = []          # (label, est_pe_ns, closure) FIFO of deferred work

            def pop_chunk():
                budget = 520
                while queue and budget > 0:
                    _, est, fn = queue.pop(0)
                    fn()
                    budget -= est

            def drain(label):
                # selective: emit only chunks with this label, preserving
                # their relative order (c1/c2 psA pairs stay adjacent)
                rest = []
                for lb, est, fn in queue:
                    if lb == label:
                        fn()
                    else:
                        rest.append((lb, est, fn))
                queue[:] = rest

            for fn in proj_chunks(0):
                fn()
            for fn in vproj_chunks(0):
                fn()
            LOOKAHEAD = 2
            for i in range(LOOKAHEAD):
                emit_qk_step(i, drain)
            for i, (hp, half, kt) in enumerate(STEPS):
                if half == 0 and kt == 0:
                    if hp + 1 < HPB:
                        queue.extend((f"proj{hp + 1}", est, fn) for est, fn in
                                     zip(proj_chunk_costs(hp + 1),
                                         proj_chunks(hp + 1)))
                    if hp == 1:
                        vc = vproj_chunks(1)
                        queue.extend(
                            ("vproj1", 0 if j == 0 else (850 if j % 2 else 900), fn)
                            for j, fn in enumerate(vc))
                    if hp == HPB - 2:
                        wo_pre[0] = prefetch_wo(0)
                    if hp == HPB - 1:
                        wo_pre[1] = prefetch_wo(1)
                emit_qk_step(i + LOOKAHEAD, drain)
                emit_av_step(i)
                pop_chunk()
            drain("vproj1")

            # ---- out projection: out[tok, DIM] = attnT.T @ Wout ----
            for n in range(DIM // 512):
                wot = wo_pre[n]
                for mt in range(NQ // P):
                    ps = psA.tile([P, 512], f32, tag="ps")
                    for k in range(KD):
                        nc.tensor.matmul(ps[:], attnT[k][:, mt * P:(mt + 1) * P],
                                         wot[k][:],
                                         start=(k == 0), stop=(k == KD - 1))
                    st = ostage.tile([P, 512], f32, tag="ost")
                    nc.vector.tensor_copy(st[:], ps[:])
                    nc.sync.dma_start(
                        out_d[mt * P:(mt + 1) * P, n * 512:(n + 1) * 512], st[:])

    nc.compile()
    return nc


def _prep_inputs(x, sin, cos, Wqkv, Wout):
    """Host-side sharding/layout prep. Returns in_maps list for 8 cores."""
    x = np.asarray(x, np.float32)
    Wqkv = np.asarray(Wqkv, np.float32)
    Wout = np.asarray(Wout, np.float32)
    scale = DH ** -0.5
    wq = (Wqkv[:, :INNER] * scale).astype(BF)
    wk = Wqkv[:, INNER:2 * INNER].astype(BF)
    wv = Wqkv[:, 2 * INNER:].astype(BF)
    wo = Wout.astype(BF)
    cos_pad, sin_pad, Rm = _build_rope_consts(
        np.asarray(sin, np.float32), np.asarray(cos, np.float32))
    rm = Rm.astype(BF)

    in_maps = []
    for c in range(NCORES):
        b, half = divmod(c, 2)
        xT = np.ascontiguousarray(x[b].T)                          # [DIM, N]
        ck, sk = cos_pad, sin_pad
        if half == 1:        # rotate tokens so this core's queries come first
            xT = np.concatenate([xT[:, NQ:], xT[:, :NQ]], axis=1)
            ck = np.concatenate([ck[:, NQ:], ck[:, :NQ]], axis=1)
            sk = np.concatenate([sk[:, NQ:], sk[:, :NQ]], axis=1)
        in_maps.append({
            "xkv": np.ascontiguousarray(xT).astype(BF),
            "wq": wq, "wk": wk, "wv": wv, "wo": wo,
            "cosk": np.ascontiguousarray(ck).astype(BF),
            "sink": np.ascontiguousarray(sk).astype(BF),
            "rm": rm,
            "tid": np.eye(P, dtype=np.float32).astype(BF),
            "par": np.array([[half]], dtype=np.int32),
        })
    return in_maps


LAST_RESULTS = None


def kernel(x, sin, cos, Wqkv, Wout):
    global LAST_RESULTS
    if "nc" not in _CACHE:
        _CACHE["nc"] = _build_program()
    nc = _CACHE["nc"]
    in_maps = _prep_inputs(x, sin, cos, Wqkv, Wout)
    trace = bool(int(os.environ.get("KERNEL_TRACE", "0")))
    try:
        res = run_bass_kernel_spmd(nc, in_maps, core_ids=list(range(NCORES)),
                                   trace=trace)
    except (ImportError, ModuleNotFoundError):
        # NTFF profiling hook unavailable in this environment
        res = run_bass_kernel_spmd(nc, in_maps, core_ids=list(range(NCORES)),
                                   trace=False)
    LAST_RESULTS = res
    out = np.empty((B, N, DIM), np.float32)
    for c in range(NCORES):
        b, half = divmod(c, 2)
        out[b, half * NQ:(half + 1) * NQ, :] = res.results[c]["out"]
    return out



# revision 8
# speedup vs baseline: 1.1410x; 1.0059x over previous
"""Trainium2 Bass kernel for nn_Attention_40492951666725.

Full attention layer: qkv proj -> RoPE (interleaved pairs, rot dim 32) ->
softmax(QK^T)V -> out proj.  B=4, N=2048, DIM=1024, H=16, DH=64.

Sharding: 8 cores, core c handles batch b=c//2 and query-half c%2 (1024
query tokens, all 16 heads, full 2048-token K/V).  The host rotates the
token axis per core so the core's own query tokens are always columns
[0:1024] of xT (attention is permutation-invariant over keys, so k/v and
the rope tables follow the same order).

K/V projection dedup: the two cores of a batch exchange halves through a
pair-shared HBM tensor (addr_space="Shared": cores 2k/2k+1 share an HBM
domain under LNC1).  Each core projects+ropes K only for its own 1024
tokens (per head-pair block, hp>=1) and V of the late feature half
(hp4-7) only for its own 8 kj tiles; cond-DMAs (on a parity input
register) write its slot, a pairwise AllGather barrier on a 4-byte token
orders the writes against the partner's reads (manual dep edges), and the
partner's half is DMA-read back just in time.  hp0's K and the early V
features stay redundant - they are needed before a 15us barrier could
resolve.

Layouts (per core):
  xT   [DIM, 2048]  (host-transposed)   -> lhsT/rhs for projections
  q^T  [feat, 1024], k^T [feat, 2048]   feat on partitions
  S^T  [kj, qi]  (kj on partitions)     -> softmax via exp (no max-sub;
        scores are O(+-10) so fp32 exp is safe), denominator from a
        ones-column appended to V, division applied per qi tile.
  AV   out[qi, 65] += P[kj,qi].T Vaug[kj, :] with P stationary: free dim
        65 instead of 512 halves tensor-engine time vs the [65, qi]
        orientation.  Accumulators live as 65-wide slices of two PSUM
        banks, pre-zeroed on DVE and accumulated with start=False (a
        plain += onto zeros) to sidestep the one-pending-accumulation-
        group-per-zero-region limit.
  attn [tok, feat] staged per qi tile, then PE-transposed (into the
        drained AV banks viewed as bf16) to attn^T [inner, tok] for the
        out projection, which produces out [tok, DIM] directly.

Schedule: attention is one flat (hp, half, kt) step stream; QK+exp run
two steps ahead of AV (psS is double-buffered, pt pool 7 deep) so the
Act engine's exp stream - the second-busiest engine - stays fed across
unit boundaries while epilogues/transposes/memsets run.  Projection and
V-projection work is chopped into ~0.5-1us chunks and spliced one per kt
into the PE stream to fill the slack left by the Act-paced exp.

RoPE: rotate_every_two(q) is a fixed feat-space linear map -> done with a
single [128,128] block-diagonal matmul (Rm), then q_rot = q*cos + (Rq)*sin
elementwise on DVE; pass-dims use cos=1/sin=0 so all 64 dims are uniform.
"""

import os
import numpy as np
import ml_dtypes

import concourse.bass as bass
from concourse.bass import _add_dep_helper
from concourse import bacc
import concourse.tile as tile
from concourse import mybir, library_config
from concourse.bass_utils import run_bass_kernel_spmd

PAIR_GROUPS = [[0, 1], [2, 3], [4, 5], [6, 7]]
i32 = mybir.dt.int32

BF = ml_dtypes.bfloat16
bf16 = mybir.dt.bfloat16
f32 = mybir.dt.float32

B, N, DIM, H, DH, ROT = 4, 2048, 1024, 16, 64, 32
INNER = H * DH
NQ = N // 2            # query tokens per core
NCORES = 8
P = 128
KD = DIM // P          # 8 contraction tiles over model dim
NKT = N // P           # 16 kj partition tiles
HPB = H // 2           # 8 head-pair blocks

Exp = mybir.ActivationFunctionType.Exp

_CACHE = {}


def _build_rope_consts(sin, cos):
    """cos_pad/sin_pad [128, N] for one head-pair feat block, Rm [128,128].

    Uses the provided sin/cos tables [N, ROT]; pass-dims get cos=1/sin=0 so
    RoPE applies uniformly over all 64 head dims."""
    cos_pad = np.ones((P, N), np.float32)
    sin_pad = np.zeros((P, N), np.float32)
    for half in range(2):                                # two heads per block
        r0 = half * DH
        cos_pad[r0:r0 + ROT, :] = cos.T
        sin_pad[r0:r0 + ROT, :] = sin.T

    # Rm[dp, d]: out[d] = sum_dp Rm[dp, d] * q[dp]  == rotate_every_two(q)[d]
    Rm = np.zeros((P, P), np.float32)
    for half in range(2):
        r0 = half * DH
        for i in range(0, ROT, 2):
            Rm[r0 + i + 1, r0 + i] = -1.0                # out[2i]   = -q[2i+1]
            Rm[r0 + i, r0 + i + 1] = 1.0                 # out[2i+1] =  q[2i]
    return cos_pad, sin_pad, Rm


def _build_program():
    nc = bacc.Bacc(trn_type="TRN2", num_devices=NCORES)

    xkv_d = nc.dram_tensor("xkv", [DIM, N], bf16, kind="ExternalInput")
    wq_d = nc.dram_tensor("wq", [DIM, INNER], bf16, kind="ExternalInput")
    wk_d = nc.dram_tensor("wk", [DIM, INNER], bf16, kind="ExternalInput")
    wv_d = nc.dram_tensor("wv", [DIM, INNER], bf16, kind="ExternalInput")
    wo_d = nc.dram_tensor("wo", [INNER, DIM], bf16, kind="ExternalInput")
    cosk_d = nc.dram_tensor("cosk", [P, N], bf16, kind="ExternalInput")
    sink_d = nc.dram_tensor("sink", [P, N], bf16, kind="ExternalInput")
    rm_d = nc.dram_tensor("rm", [P, P], bf16, kind="ExternalInput")
    tid_d = nc.dram_tensor("tid", [P, P], bf16, kind="ExternalInput")
    par_d = nc.dram_tensor("par", [1, 1], i32, kind="ExternalInput")
    out_d = nc.dram_tensor("out", [NQ, DIM], f32, kind="ExternalOutput")
    # Pair-shared HBM staging for the K/V halves exchanged between the two
    # cores of a batch (cores 2k, 2k+1 share an HBM domain under LNC1).
    ksh_d = nc.dram_tensor("ksh", [2, HPB, P, NQ], bf16, kind="Internal",
                           addr_space="Shared")
    vsh_d = nc.dram_tensor("vsh", [2, HPB, P, 520], bf16, kind="Internal",
                           addr_space="Shared")
    bar_in_d = nc.dram_tensor("bar_in", [1, 1], f32, kind="Internal")
    bar_out_d = nc.dram_tensor("bar_out", [1, 2], f32, kind="Internal")

    with tile.TileContext(nc) as tc:
        with (
            tc.tile_pool(name="res", bufs=1) as res,          # kernel-lifetime tiles
            tc.tile_pool(name="kstream", bufs=2) as kstream,  # per-hp q/k tiles
            tc.tile_pool(name="wstream", bufs=1) as wstream,
            tc.tile_pool(name="pt", bufs=7) as ptp,           # P^T tiles
            tc.tile_pool(name="tmp", bufs=4) as tmp,          # rope DVE temps
            tc.tile_pool(name="small", bufs=2) as small,
            tc.tile_pool(name="ostage", bufs=2) as ostage,
            tc.tile_pool(name="astage", bufs=2) as astage,
            tc.tile_pool(name="psA", bufs=2, space="PSUM") as psA,    # [128,512] proj/outproj/transp
            tc.tile_pool(name="psS", bufs=2, space="PSUM") as psS,    # [128,1024] scores
            tc.tile_pool(name="psV", bufs=1, space="PSUM") as psV,    # 2 banks, 4 AV accums each
        ):
            nc.gpsimd.load_library(library_config.attn)

            # ---- pair-exchange plumbing: parity register + barrier helper ----
            par_sb = res.tile([1, 1], i32, tag="par", name="par_sb")
            nc.sync.dma_start(par_sb[:], par_d[:])
            zz = res.tile([1, 1], f32, tag="zz", name="zz")
            nc.vector.memset(zz[:], 0.0)
            nc.sync.dma_start(bar_in_d[:], zz[:])
            par_reg = nc.sync.alloc_register("par_reg")
            nc.sync.reg_load(par_reg, par_sb[0:1, 0:1])
            par = nc.sync.snap(par_reg, donate=True, min_val=0, max_val=1)

            def pair_exchange(writes_fn, reads_fn):
                """SPMD pair exchange: cond-write my slot, barrier, cond-read
                the partner's slot.  writes_fn/reads_fn(slot) emit the DMAs
                for a given shared-HBM slot index."""
                w_mine = writes_fn(0, par == 0) + writes_fn(1, par == 1)
                barr = nc.gpsimd.collective_compute(
                    "AllGather", mybir.AluOpType.bypass, PAIR_GROUPS,
                    ins=[bar_in_d[0:1, 0:1]], outs=[bar_out_d[0:1, 0:2]])
                for w in w_mine:
                    _add_dep_helper(barr.ins, w.ins, sync=True,
                                    reason="pair barrier waits for my writes")
                r_mine = reads_fn(1, par == 0) + reads_fn(0, par == 1)
                for r in r_mine:
                    _add_dep_helper(r.ins, barr.ins, sync=True,
                                    reason="partner reads gated on barrier")

            # ---- resident loads, ordered so proj(0) starts ASAP:
            # hp=0 only needs wq/wk cols 0:128, Q blocks only xkv cols 0:NQ.
            cosk = res.tile([P, N], bf16, tag="cosk")
            sink = res.tile([P, N], bf16, tag="sink")
            rm = res.tile([P, P], bf16, tag="rm")
            ident = res.tile([P, P], bf16, tag="tid", name="tid_sb")
            for t, d in ((rm, rm_d), (ident, tid_d)):
                nc.sync.dma_start(t[:], d[:])
            xkv, wq, wk = [], [], []
            for k in range(KD):
                t = res.tile([P, N], bf16, tag=f"xkv{k}", name=f"xkv{k}")
                xkv.append(t)
                w = res.tile([P, DIM], bf16, tag=f"wq{k}", name=f"wq{k}")
                wq.append(w)
                nc.sync.dma_start(t[:, 0:NQ], xkv_d[k * P:(k + 1) * P, 0:NQ])
                nc.sync.dma_start(w[:], wq_d[k * P:(k + 1) * P, :])
                if k == 3:
                    for ct, d in ((cosk, cosk_d), (sink, sink_d)):
                        nc.sync.dma_start(ct[:], d[:])
            for k in range(KD):
                t = res.tile([P, DIM], bf16, tag=f"wk{k}", name=f"wk{k}")
                wk.append(t)
                nc.sync.dma_start(t[:], wk_d[k * P:(k + 1) * P, :])
            for k in range(KD):
                nc.sync.dma_start(xkv[k][:, NQ:N], xkv_d[k * P:(k + 1) * P, NQ:N])

            attnT = []
            for k in range(KD):
                attnT.append(res.tile([P, NQ], bf16, tag=f"attnT{k}", name=f"attnT{k}"))
            vaug = []
            for mt in range(NKT):
                vt = res.tile([P, HPB, 2, 65], bf16, tag=f"vaug{mt}", name=f"vaug{mt}")
                nc.vector.memset(vt[:, :, :, 64], 1.0)
                vaug.append(vt)

            def vproj_chunks(bn):
                """Chunked V projection of feats [bn*512, (bn+1)*512)."""
                wvt = []

                def cdma():
                    for k in range(KD):
                        t = wstream.tile([P, 512], bf16, tag=f"wv{k}", name=f"wv{k}")
                        nc.sync.dma_start(t[:], wv_d[k * P:(k + 1) * P,
                                                       bn * 512:(bn + 1) * 512])
                        wvt.append(t)

                # bn=0 (features for hp0-3, needed early) is projected
                # redundantly for all 16 token tiles; bn=1 (hp4-7, needed
                # ~150us in) only for the core's own 8 tiles, with the other 8
                # arriving from the pair core via shared HBM.
                nmt = NKT if bn == 0 else NKT // 2
                chunks = [cdma]
                for mt in range(nmt):
                    box = {}

                    def c1(mt=mt):
                        ps = psA.tile([P, 512], f32, tag="ps", name="ps")
                        box[0] = ps
                        for k in range(4):
                            nc.tensor.matmul(ps[:], xkv[k][:, mt * P:(mt + 1) * P],
                                             wvt[k][:],
                                             start=(k == 0), stop=False)

                    def c2(mt=mt):
                        ps = box[0]
                        for k in range(4, KD):
                            nc.tensor.matmul(ps[:], xkv[k][:, mt * P:(mt + 1) * P],
                                             wvt[k][:],
                                             start=False, stop=(k == KD - 1))
                        nc.vector.tensor_copy(
                            vaug[mt][:, bn * 4:(bn + 1) * 4, :, 0:64],
                            ps[:].rearrange("p (b h d) -> p b h d", b=4, h=2))

                    chunks += [c1, c2]
                if bn == 1:
                    def cx():
                        pair_exchange(
                            lambda slot, cond: [
                                nc.sync.dma_start(vsh_d[slot, mt],
                                                    vaug[mt][:, 4:8, :, :],
                                                    cond=cond)
                                for mt in range(NKT // 2)],
                            lambda slot, cond: [
                                nc.sync.dma_start(vaug[mt + NKT // 2][:, 4:8, :, :],
                                                    vsh_d[slot, mt],
                                                    cond=cond)
                                for mt in range(NKT // 2)])
                    chunks.append(cx)
                return chunks

            state = {}

            def proj_block_chunks(dst, w, c0, n):
                """Two chunks: 4+4 matmuls accumulating one 512-token block.

                The psA 'ps' tag has bufs=2 and every chunk pair is adjacent
                in the FIFO, so the accumulator survives until its second
                chunk (at most one other 'ps' alloc in between)."""
                box = {}

                def c1():
                    ps = psA.tile([P, 512], f32, tag="ps", name="ps")
                    box[0] = ps
                    for k in range(4):
                        nc.tensor.matmul(ps[:], w[k][:, c0:c0 + P],
                                         xkv[k][:, n * 512:(n + 1) * 512],
                                         start=(k == 0), stop=False)

                def c2():
                    ps = box[0]
                    for k in range(4, KD):
                        nc.tensor.matmul(ps[:], w[k][:, c0:c0 + P],
                                         xkv[k][:, n * 512:(n + 1) * 512],
                                         start=False, stop=(k == KD - 1))
                    nc.vector.tensor_copy(dst[:, n * 512:(n + 1) * 512], ps[:])

                return [c1, c2]

            def rope_chunk(dst, n):
                sl = slice(n * 512, (n + 1) * 512)

                def c3():
                    psw = psA.tile([P, 512], f32, tag="ps", name="psw")
                    nc.tensor.matmul(psw[:], rm[:], dst[:, sl], start=True, stop=True)
                    t1 = tmp.tile([P, 512], bf16, tag="t1", name="t1")
                    nc.vector.tensor_mul(t1[:], dst[:, sl], cosk[:, sl])
                    t2 = tmp.tile([P, 512], bf16, tag="t2", name="t2")
                    nc.vector.tensor_mul(t2[:], psw[:], sink[:, sl])
                    nc.vector.tensor_add(dst[:, sl], t1[:], t2[:])

                return [c3]

            def proj_chunks(hp):
                """Chunked projection+rope of feat block hp (heads 2hp, 2hp+1).

                For hp >= 1 only the core's own token half of K is projected
                and rope'd; the other half arrives rope'd from the pair core
                via shared HBM (hp0 stays redundant: its kt8-15 are needed
                ~17us in, before a 15us barrier could resolve)."""
                c0 = hp * P
                qraw = kstream.tile([P, NQ], bf16, tag="qraw", name="qraw")
                kraw = kstream.tile([P, N], bf16, tag="kraw", name="kraw")
                state[hp] = (qraw, kraw)
                chunks = []
                for n in range(NQ // 512):
                    chunks += proj_block_chunks(qraw, wq, c0, n)
                    chunks += rope_chunk(qraw, n)
                khalf = N if hp == 0 else NQ
                for n in range(khalf // 512):
                    chunks += proj_block_chunks(kraw, wk, c0, n)
                    chunks += rope_chunk(kraw, n)
                if hp > 0:
                    def cx():
                        pair_exchange(
                            lambda slot, cond: [nc.sync.dma_start(
                                ksh_d[slot, hp], kraw[:, 0:NQ], cond=cond)],
                            lambda slot, cond: [nc.sync.dma_start(
                                kraw[:, NQ:N], ksh_d[slot, hp], cond=cond)])
                    chunks.append(cx)
                return chunks

            def proj_chunk_costs(hp):
                qk_blocks = NQ // 512 + (N if hp == 0 else NQ) // 512
                costs = [850, 900, 250] * qk_blocks
                if hp > 0:
                    costs.append(0)
                return costs

            # ---- attention as one flat (hp, half, kt) step stream.  The QK+exp
            # side runs LOOKAHEAD steps ahead of the AV side, crossing unit
            # boundaries, so the Act engine's exp stream never drains while
            # epilogues/transposes/memsets run between units.  AV uses P as
            # stationary (free dim 65 vs 512, halving tensor-engine time); its
            # psum banks are pre-zeroed on DVE and accumulated with
            # start=False -- hardware-wise a plain += onto zeros -- to
            # sidestep the one-pending-group-per-zero-region limit. ----
            STEPS = [(hp, half, kt) for hp in range(HPB) for half in (0, 1)
                     for kt in range(NKT)]
            pts = {}
            stages = {}
            avctx = {}

            def emit_qk_step(idx, drain):
                if idx >= len(STEPS):
                    return
                hp, half, kt = STEPS[idx]
                if half == 0 and kt == 0:
                    drain(f"proj{hp}")      # qraw/kraw for hp must be complete
                qrot, krot = state[hp]
                hoff = half * DH
                ps = psS.tile([P, NQ], f32, tag="s", name="s")
                for qn in range(NQ // 512):
                    nc.tensor.matmul(
                        ps[:, qn * 512:(qn + 1) * 512],
                        krot[hoff:hoff + DH, kt * P:(kt + 1) * P],
                        qrot[hoff:hoff + DH, qn * 512:(qn + 1) * 512],
                        start=True, stop=True)
                pt = ptp.tile([P, NQ], bf16, tag="pt", name="pt")
                nc.scalar.activation(pt[:], ps[:], Exp)
                pts[idx] = pt

            def emit_av_step(idx):
                hp, half, kt = STEPS[idx]
                hoff = half * DH
                if kt == 0:
                    if half == 0:
                        stages[hp] = [astage.tile([P, P], bf16, tag=f"st{qt}",
                                                  name=f"st{qt}")
                                      for qt in range(NQ // P)]
                    banks = [psV.tile([P, 512], f32, tag=f"bank{i}",
                                      name=f"bank{i}") for i in range(2)]
                    for b in banks:
                        nc.vector.memset(b[:], 0.0)
                    avctx[(hp, half)] = banks
                banks = avctx[(hp, half)]
                pvs = [banks[qt // 4][:, (qt % 4) * P:(qt % 4) * P + 65]
                       for qt in range(NQ // P)]
                pt = pts.pop(idx)
                for qt in range(NQ // P):
                    nc.tensor.matmul(pvs[qt],
                                     pt[:, qt * P:(qt + 1) * P],
                                     vaug[kt][:, hp, half, :],
                                     start=False, stop=(kt == NKT - 1),
                                     skip_group_check=True)
                if kt == NKT - 1:
                    stage = stages[hp]
                    for qt in range(NQ // P):
                        pv = pvs[qt]
                        rec = small.tile([P, 1], f32, tag="rec", name="rec")
                        nc.vector.reciprocal(rec[:], pv[:, 64:65])
                        nc.vector.tensor_scalar_mul(
                            stage[qt][:, hoff:hoff + DH], pv[:, 0:64], rec[:])
                    if half == 1:
                        emit_attn_transpose(hp, stages.pop(hp), banks)
                        avctx.pop((hp, 0))
                        avctx.pop((hp, 1))
                        state.pop(hp)

            def emit_attn_transpose(hp, stage, banks):
                """stage[qt] [tok 128, feat 128] -> attnT[hp] [feat, tok].

                Transposes land in the (just-drained) AV psum banks, viewed
                as bf16, so no extra PSUM bank or psA tag is needed."""
                for qt in range(NQ // P):
                    tr = banks[qt // 4][:, (qt % 4) * P:(qt % 4) * P + 64].bitcast(bf16)
                    nc.tensor.transpose(tr, stage[qt][:], ident[:])
                    nc.vector.tensor_copy(attnT[hp][:, qt * P:(qt + 1) * P], tr)

            def prefetch_wo(n):
                wot = []
                for k in range(KD):
                    t = wstream.tile([P, 512], bf16, tag=f"wo{n}_{k}",
                                     name=f"wo{n}_{k}", bufs=1)
                    nc.sync.dma_start(t[:], wo_d[k * P:(k + 1) * P,
                                                   n * 512:(n + 1) * 512])
                    wot.append(t)
                return wot

            # ---- main loop: attention per (hp, half), with next-hp projection
            # work spliced chunk-by-chunk into the per-kt PE slack ----
            wo_pre = {}
            queue = []          # (label, est_pe_ns, closure) FIFO of deferred work

            def pop_chunk():
                # vproj0 feeds the same-window AV steps of hp0: drain it at
                # double rate so vaug[kt] is ready when av(kt) needs it.
                budget = 1800 if (queue and queue[0][0] == "vproj0") else 520
                while queue and budget > 0:
                    _, est, fn = queue.pop(0)
                    fn()
                    budget -= est

            def drain(label):
                # selective: emit only chunks with this label, preserving
                # their relative order (c1/c2 psA pairs stay adjacent)
                rest = []
                for lb, est, fn in queue:
                    if lb == label:
                        fn()
                    else:
                        rest.append((lb, est, fn))
                queue[:] = rest

            for fn in proj_chunks(0):
                fn()
            queue.extend(("vproj0", 0 if j == 0 else (850 if j % 2 else 900), fn)
                         for j, fn in enumerate(vproj_chunks(0)))
            LOOKAHEAD = 2
            for i in range(LOOKAHEAD):
                emit_qk_step(i, drain)
            for i, (hp, half, kt) in enumerate(STEPS):
                if half == 0 and kt == 0:
                    if hp + 1 < HPB:
                        queue.extend((f"proj{hp + 1}", est, fn) for est, fn in
                                     zip(proj_chunk_costs(hp + 1),
                                         proj_chunks(hp + 1)))
                    if hp == 1:
                        vc = vproj_chunks(1)
                        queue.extend(
                            ("vproj1", 0 if j == 0 else (850 if j % 2 else 900), fn)
                            for j, fn in enumerate(vc))
                    if hp == HPB - 2:
                        wo_pre[0] = prefetch_wo(0)
                    if hp == HPB - 1:
                        wo_pre[1] = prefetch_wo(1)
                emit_qk_step(i + LOOKAHEAD, drain)
                pop_chunk()     # before AV: hp0's AV needs same-window vproj0
                emit_av_step(i)
            drain("vproj1")

            # ---- out projection: out[tok, DIM] = attnT.T @ Wout ----
            for n in range(DIM // 512):
                wot = wo_pre[n]
                for mt in range(NQ // P):
                    ps = psA.tile([P, 512], f32, tag="ps")
                    for k in range(KD):
                        nc.tensor.matmul(ps[:], attnT[k][:, mt * P:(mt + 1) * P],
                                         wot[k][:],
                                         start=(k == 0), stop=(k == KD - 1))
                    st = ostage.tile([P, 512], f32, tag="ost")
                    nc.vector.tensor_copy(st[:], ps[:])
                    nc.sync.dma_start(
                        out_d[mt * P:(mt + 1) * P, n * 512:(n + 1) * 512], st[:])

    nc.compile()
    return nc


def _prep_inputs(x, sin, cos, Wqkv, Wout):
    """Host-side sharding/layout prep. Returns in_maps list for 8 cores."""
    x = np.asarray(x, np.float32)
    Wqkv = np.asarray(Wqkv, np.float32)
    Wout = np.asarray(Wout, np.float32)
    scale = DH ** -0.5
    wq = (Wqkv[:, :INNER] * scale).astype(BF)
    wk = Wqkv[:, INNER:2 * INNER].astype(BF)
    wv = Wqkv[:, 2 * INNER:].astype(BF)
    wo = Wout.astype(BF)
    cos_pad, sin_pad, Rm = _build_rope_consts(
        np.asarray(sin, np.float32), np.asarray(cos, np.float32))
    rm = Rm.astype(BF)

    in_maps = []
    for c in range(NCORES):
        b, half = divmod(c, 2)
        xT = np.ascontiguousarray(x[b].T)                          # [DIM, N]
        ck, sk = cos_pad, sin_pad
        if half == 1:        # rotate tokens so this core's queries come first
            xT = np.concatenate([xT[:, NQ:], xT[:, :NQ]], axis=1)
            ck = np.concatenate([ck[:, NQ:], ck[:, :NQ]], axis=1)
            sk = np.concatenate([sk[:, NQ:], sk[:, :NQ]], axis=1)
        in_maps.append({
            "xkv": np.ascontiguousarray(xT).astype(BF),
            "wq": wq, "wk": wk, "wv": wv, "wo": wo,
            "cosk": np.ascontiguousarray(ck).astype(BF),
            "sink": np.ascontiguousarray(sk).astype(BF),
            "rm": rm,
            "tid": np.eye(P, dtype=np.float32).astype(BF),
            "par": np.array([[half]], dtype=np.int32),
        })
    return in_maps


LAST_RESULTS = None


def kernel(x, sin, cos, Wqkv, Wout):
    global LAST_RESULTS
    if "nc" not in _CACHE:
        _CACHE["nc"] = _build_program()
    nc = _CACHE["nc"]
    in_maps = _prep_inputs(x, sin, cos, Wqkv, Wout)
    trace = bool(int(os.environ.get("KERNEL_TRACE", "0")))
    try:
        res = run_bass_kernel_spmd(nc, in_maps, core_ids=list(range(NCORES)),
                                   trace=trace)
    except (ImportError, ModuleNotFoundError):
        # NTFF profiling hook unavailable in this environment
        res = run_bass_kernel_spmd(nc, in_maps, core_ids=list(range(NCORES)),
                                   trace=False)
    LAST_RESULTS = res
    out = np.empty((B, N, DIM), np.float32)
    for c in range(NCORES):
        b, half = divmod(c, 2)
        out[b, half * NQ:(half + 1) * NQ, :] = res.results[c]["out"]
    return out



# revision 9
# speedup vs baseline: 1.1439x; 1.0025x over previous
"""Trainium2 Bass kernel for nn_Attention_40492951666725.

Full attention layer: qkv proj -> RoPE (interleaved pairs, rot dim 32) ->
softmax(QK^T)V -> out proj.  B=4, N=2048, DIM=1024, H=16, DH=64.

Sharding: 8 cores, core c handles batch b=c//2 and query-half c%2 (1024
query tokens, all 16 heads, full 2048-token K/V).  The host rotates the
token axis per core so the core's own query tokens are always columns
[0:1024] of xT (attention is permutation-invariant over keys, so k/v and
the rope tables follow the same order).

K/V projection dedup: the two cores of a batch exchange halves through a
pair-shared HBM tensor (addr_space="Shared": cores 2k/2k+1 share an HBM
domain under LNC1).  Each core projects+ropes K only for its own 1024
tokens (per head-pair block, hp>=1) and V of the late feature half
(hp4-7) only for its own 8 kj tiles; cond-DMAs (on a parity input
register) write its slot, a pairwise AllGather barrier on a 4-byte token
orders the writes against the partner's reads (manual dep edges), and the
partner's half is DMA-read back just in time.  hp0's K and the early V
features stay redundant - they are needed before a 15us barrier could
resolve.

Layouts (per core):
  xT   [DIM, 2048]  (host-transposed)   -> lhsT/rhs for projections
  q^T  [feat, 1024], k^T [feat, 2048]   feat on partitions
  S^T  [kj, qi]  (kj on partitions)     -> softmax via exp (no max-sub;
        scores are O(+-10) so fp32 exp is safe), denominator from a
        ones-column appended to V, division applied per qi tile.
  AV   out[qi, 65] += P[kj,qi].T Vaug[kj, :] with P stationary: free dim
        65 instead of 512 halves tensor-engine time vs the [65, qi]
        orientation.  Accumulators live as 65-wide slices of two PSUM
        banks, pre-zeroed on DVE and accumulated with start=False (a
        plain += onto zeros) to sidestep the one-pending-accumulation-
        group-per-zero-region limit.
  attn [tok, feat] staged per qi tile, then PE-transposed (into the
        drained AV banks viewed as bf16) to attn^T [inner, tok] for the
        out projection, which produces out [tok, DIM] directly.

Schedule: attention is one flat (hp, half, kt) step stream; QK+exp run
two steps ahead of AV (psS is double-buffered, pt pool 7 deep) so the
Act engine's exp stream - the second-busiest engine - stays fed across
unit boundaries while epilogues/transposes/memsets run.  Projection and
V-projection work is chopped into ~0.5-1us chunks and spliced one per kt
into the PE stream to fill the slack left by the Act-paced exp.

RoPE: rotate_every_two(q) is a fixed feat-space linear map -> done with a
single [128,128] block-diagonal matmul (Rm), then q_rot = q*cos + (Rq)*sin
elementwise on DVE; pass-dims use cos=1/sin=0 so all 64 dims are uniform.
"""

import os
import numpy as np
import ml_dtypes

import concourse.bass as bass
from concourse.bass import _add_dep_helper
from concourse import bacc
import concourse.tile as tile
from concourse import mybir, library_config
from concourse.bass_utils import run_bass_kernel_spmd

PAIR_GROUPS = [[0, 1], [2, 3], [4, 5], [6, 7]]
i32 = mybir.dt.int32

BF = ml_dtypes.bfloat16
bf16 = mybir.dt.bfloat16
f32 = mybir.dt.float32

B, N, DIM, H, DH, ROT = 4, 2048, 1024, 16, 64, 32
INNER = H * DH
NQ = N // 2            # query tokens per core
NCORES = 8
P = 128
KD = DIM // P          # 8 contraction tiles over model dim
NKT = N // P           # 16 kj partition tiles
HPB = H // 2           # 8 head-pair blocks

Exp = mybir.ActivationFunctionType.Exp

_CACHE = {}


def _build_rope_consts(sin, cos):
    """cos_pad/sin_pad [128, N] for one head-pair feat block, Rm [128,128].

    Uses the provided sin/cos tables [N, ROT]; pass-dims get cos=1/sin=0 so
    RoPE applies uniformly over all 64 head dims."""
    cos_pad = np.ones((P, N), np.float32)
    sin_pad = np.zeros((P, N), np.float32)
    for half in range(2):                                # two heads per block
        r0 = half * DH
        cos_pad[r0:r0 + ROT, :] = cos.T
        sin_pad[r0:r0 + ROT, :] = sin.T

    # Rm[dp, d]: out[d] = sum_dp Rm[dp, d] * q[dp]  == rotate_every_two(q)[d]
    Rm = np.zeros((P, P), np.float32)
    for half in range(2):
        r0 = half * DH
        for i in range(0, ROT, 2):
            Rm[r0 + i + 1, r0 + i] = -1.0                # out[2i]   = -q[2i+1]
            Rm[r0 + i, r0 + i + 1] = 1.0                 # out[2i+1] =  q[2i]
    return cos_pad, sin_pad, Rm


def _build_program():
    nc = bacc.Bacc(trn_type="TRN2", num_devices=NCORES)

    xkv_d = nc.dram_tensor("xkv", [DIM, N], bf16, kind="ExternalInput")
    wq_d = nc.dram_tensor("wq", [DIM, INNER], bf16, kind="ExternalInput")
    wk_d = nc.dram_tensor("wk", [DIM, INNER], bf16, kind="ExternalInput")
    wv_d = nc.dram_tensor("wv", [DIM, INNER], bf16, kind="ExternalInput")
    wo_d = nc.dram_tensor("wo", [INNER, DIM], bf16, kind="ExternalInput")
    cosk_d = nc.dram_tensor("cosk", [P, N], bf16, kind="ExternalInput")
    sink_d = nc.dram_tensor("sink", [P, N], bf16, kind="ExternalInput")
    rm_d = nc.dram_tensor("rm", [P, P], bf16, kind="ExternalInput")
    tid_d = nc.dram_tensor("tid", [P, P], bf16, kind="ExternalInput")
    par_d = nc.dram_tensor("par", [1, 1], i32, kind="ExternalInput")
    out_d = nc.dram_tensor("out", [NQ, DIM], f32, kind="ExternalOutput")
    # Pair-shared HBM staging for the K/V halves exchanged between the two
    # cores of a batch (cores 2k, 2k+1 share an HBM domain under LNC1).
    ksh_d = nc.dram_tensor("ksh", [2, HPB, P, NQ], bf16, kind="Internal",
                           addr_space="Shared")
    vsh_d = nc.dram_tensor("vsh", [2, HPB, P, 520], bf16, kind="Internal",
                           addr_space="Shared")
    bar_in_d = nc.dram_tensor("bar_in", [1, 1], f32, kind="Internal")
    bar_out_d = nc.dram_tensor("bar_out", [1, 2], f32, kind="Internal")

    with tile.TileContext(nc) as tc:
        with (
            tc.tile_pool(name="res", bufs=1) as res,          # kernel-lifetime tiles
            tc.tile_pool(name="kstream", bufs=2) as kstream,  # per-hp q/k tiles
            tc.tile_pool(name="wstream", bufs=1) as wstream,
            tc.tile_pool(name="pt", bufs=7) as ptp,           # P^T tiles
            tc.tile_pool(name="tmp", bufs=4) as tmp,          # rope DVE temps
            tc.tile_pool(name="small", bufs=2) as small,
            tc.tile_pool(name="ostage", bufs=2) as ostage,
            tc.tile_pool(name="astage", bufs=2) as astage,
            tc.tile_pool(name="psA", bufs=2, space="PSUM") as psA,    # [128,512] proj/outproj/transp
            tc.tile_pool(name="psS", bufs=2, space="PSUM") as psS,    # [128,1024] scores
            tc.tile_pool(name="psV", bufs=1, space="PSUM") as psV,    # 2 banks, 4 AV accums each
        ):
            nc.gpsimd.load_library(library_config.attn)

            # ---- pair-exchange plumbing: parity register + barrier helper ----
            par_sb = res.tile([1, 1], i32, tag="par", name="par_sb")
            nc.sync.dma_start(par_sb[:], par_d[:])
            zz = res.tile([1, 1], f32, tag="zz", name="zz")
            nc.vector.memset(zz[:], 0.0)
            nc.sync.dma_start(bar_in_d[:], zz[:])
            par_reg = nc.sync.alloc_register("par_reg")
            nc.sync.reg_load(par_reg, par_sb[0:1, 0:1])
            par = nc.sync.snap(par_reg, donate=True, min_val=0, max_val=1)

            def pair_exchange(writes_fn, reads_fn):
                """SPMD pair exchange: cond-write my slot, barrier, cond-read
                the partner's slot.  writes_fn/reads_fn(slot) emit the DMAs
                for a given shared-HBM slot index."""
                w_mine = writes_fn(0, par == 0) + writes_fn(1, par == 1)
                barr = nc.gpsimd.collective_compute(
                    "AllGather", mybir.AluOpType.bypass, PAIR_GROUPS,
                    ins=[bar_in_d[0:1, 0:1]], outs=[bar_out_d[0:1, 0:2]])
                for w in w_mine:
                    _add_dep_helper(barr.ins, w.ins, sync=True,
                                    reason="pair barrier waits for my writes")
                r_mine = reads_fn(1, par == 0) + reads_fn(0, par == 1)
                for r in r_mine:
                    _add_dep_helper(r.ins, barr.ins, sync=True,
                                    reason="partner reads gated on barrier")

            # ---- resident loads, ordered so proj(0) starts ASAP:
            # hp=0 only needs wq/wk cols 0:128, Q blocks only xkv cols 0:NQ.
            cosk = res.tile([P, N], bf16, tag="cosk")
            sink = res.tile([P, N], bf16, tag="sink")
            rm = res.tile([P, P], bf16, tag="rm")
            ident = res.tile([P, P], bf16, tag="tid", name="tid_sb")
            for t, d in ((rm, rm_d), (ident, tid_d)):
                nc.sync.dma_start(t[:], d[:])
            xkv, wq, wk = [], [], []
            for k in range(KD):
                t = res.tile([P, N], bf16, tag=f"xkv{k}", name=f"xkv{k}")
                xkv.append(t)
                w = res.tile([P, DIM], bf16, tag=f"wq{k}", name=f"wq{k}")
                wq.append(w)
                nc.sync.dma_start(t[:, 0:NQ], xkv_d[k * P:(k + 1) * P, 0:NQ])
                nc.sync.dma_start(w[:], wq_d[k * P:(k + 1) * P, :])
                if k == 3:
                    for ct, d in ((cosk, cosk_d), (sink, sink_d)):
                        nc.sync.dma_start(ct[:], d[:])
            for k in range(KD):
                t = res.tile([P, DIM], bf16, tag=f"wk{k}", name=f"wk{k}")
                wk.append(t)
                nc.sync.dma_start(t[:], wk_d[k * P:(k + 1) * P, :])
            for k in range(KD):
                nc.sync.dma_start(xkv[k][:, NQ:N], xkv_d[k * P:(k + 1) * P, NQ:N])

            attnT = []
            for k in range(KD):
                attnT.append(res.tile([P, NQ], bf16, tag=f"attnT{k}", name=f"attnT{k}"))
            vaug = []
            for mt in range(NKT):
                vt = res.tile([P, HPB, 2, 65], bf16, tag=f"vaug{mt}", name=f"vaug{mt}")
                nc.vector.memset(vt[:, :, :, 64], 1.0)
                vaug.append(vt)

            def vproj_chunks(bn):
                """Chunked V projection of feats [bn*512, (bn+1)*512)."""
                wvt = []

                def cdma():
                    for k in range(KD):
                        t = wstream.tile([P, 512], bf16, tag=f"wv{k}", name=f"wv{k}")
                        nc.sync.dma_start(t[:], wv_d[k * P:(k + 1) * P,
                                                       bn * 512:(bn + 1) * 512])
                        wvt.append(t)

                # bn=0 (features for hp0-3, needed early) is projected
                # redundantly for all 16 token tiles; bn=1 (hp4-7, needed
                # ~150us in) only for the core's own 8 tiles, with the other 8
                # arriving from the pair core via shared HBM.
                nmt = NKT if bn == 0 else NKT // 2
                chunks = [cdma]
                for mt in range(nmt):
                    box = {}

                    def c1(mt=mt):
                        ps = psA.tile([P, 512], f32, tag="ps", name="ps")
                        box[0] = ps
                        for k in range(4):
                            nc.tensor.matmul(ps[:], xkv[k][:, mt * P:(mt + 1) * P],
                                             wvt[k][:],
                                             start=(k == 0), stop=False)

                    def c2(mt=mt):
                        ps = box[0]
                        for k in range(4, KD):
                            nc.tensor.matmul(ps[:], xkv[k][:, mt * P:(mt + 1) * P],
                                             wvt[k][:],
                                             start=False, stop=(k == KD - 1))
                        nc.vector.tensor_copy(
                            vaug[mt][:, bn * 4:(bn + 1) * 4, :, 0:64],
                            ps[:].rearrange("p (b h d) -> p b h d", b=4, h=2))

                    chunks += [c1, c2]
                if bn == 1:
                    def cx():
                        pair_exchange(
                            lambda slot, cond: [
                                nc.sync.dma_start(vsh_d[slot, mt],
                                                    vaug[mt][:, 4:8, :, :],
                                                    cond=cond)
                                for mt in range(NKT // 2)],
                            lambda slot, cond: [
                                nc.sync.dma_start(vaug[mt + NKT // 2][:, 4:8, :, :],
                                                    vsh_d[slot, mt],
                                                    cond=cond)
                                for mt in range(NKT // 2)])
                    chunks.append(cx)
                return chunks

            state = {}

            def proj_block_chunks(dst, w, c0, n):
                """Two chunks: 4+4 matmuls accumulating one 512-token block.

                The psA 'ps' tag has bufs=2 and every chunk pair is adjacent
                in the FIFO, so the accumulator survives until its second
                chunk (at most one other 'ps' alloc in between)."""
                box = {}

                def c1():
                    ps = psA.tile([P, 512], f32, tag="ps", name="ps")
                    box[0] = ps
                    for k in range(4):
                        nc.tensor.matmul(ps[:], w[k][:, c0:c0 + P],
                                         xkv[k][:, n * 512:(n + 1) * 512],
                                         start=(k == 0), stop=False)

                def c2():
                    ps = box[0]
                    for k in range(4, KD):
                        nc.tensor.matmul(ps[:], w[k][:, c0:c0 + P],
                                         xkv[k][:, n * 512:(n + 1) * 512],
                                         start=False, stop=(k == KD - 1))
                    nc.vector.tensor_copy(dst[:, n * 512:(n + 1) * 512], ps[:])

                return [c1, c2]

            def rope_chunk(dst, n):
                sl = slice(n * 512, (n + 1) * 512)

                def c3():
                    psw = psA.tile([P, 512], f32, tag="ps", name="psw")
                    nc.tensor.matmul(psw[:], rm[:], dst[:, sl], start=True, stop=True)
                    t1 = tmp.tile([P, 512], bf16, tag="t1", name="t1")
                    nc.vector.tensor_mul(t1[:], dst[:, sl], cosk[:, sl])
                    t2 = tmp.tile([P, 512], bf16, tag="t2", name="t2")
                    nc.vector.tensor_mul(t2[:], psw[:], sink[:, sl])
                    nc.vector.tensor_add(dst[:, sl], t1[:], t2[:])

                return [c3]

            def proj_chunks(hp):
                """Chunked projection+rope of feat block hp (heads 2hp, 2hp+1).

                For hp >= 1 only the core's own token half of K is projected
                and rope'd; the other half arrives rope'd from the pair core
                via shared HBM (hp0 stays redundant: its kt8-15 are needed
                ~17us in, before a 15us barrier could resolve)."""
                c0 = hp * P
                qraw = kstream.tile([P, NQ], bf16, tag="qraw", name="qraw")
                kraw = kstream.tile([P, N], bf16, tag="kraw", name="kraw")
                state[hp] = (qraw, kraw)
                chunks = []
                for n in range(NQ // 512):
                    chunks += proj_block_chunks(qraw, wq, c0, n)
                    chunks += rope_chunk(qraw, n)
                khalf = N if hp == 0 else NQ
                for n in range(khalf // 512):
                    chunks += proj_block_chunks(kraw, wk, c0, n)
                    chunks += rope_chunk(kraw, n)
                if hp > 0:
                    def cx():
                        pair_exchange(
                            lambda slot, cond: [nc.sync.dma_start(
                                ksh_d[slot, hp], kraw[:, 0:NQ], cond=cond)],
                            lambda slot, cond: [nc.sync.dma_start(
                                kraw[:, NQ:N], ksh_d[slot, hp], cond=cond)])
                    chunks.append(cx)
                return chunks

            def proj_chunk_costs(hp):
                qk_blocks = NQ // 512 + (N if hp == 0 else NQ) // 512
                costs = [850, 900, 250] * qk_blocks
                if hp > 0:
                    costs.append(0)
                return costs

            # ---- attention as one flat (hp, half, kt) step stream.  The QK+exp
            # side runs LOOKAHEAD steps ahead of the AV side, crossing unit
            # boundaries, so the Act engine's exp stream never drains while
            # epilogues/transposes/memsets run between units.  AV uses P as
            # stationary (free dim 65 vs 512, halving tensor-engine time); its
            # psum banks are pre-zeroed on DVE and accumulated with
            # start=False -- hardware-wise a plain += onto zeros -- to
            # sidestep the one-pending-group-per-zero-region limit. ----
            STEPS = [(hp, half, kt) for hp in range(HPB) for half in (0, 1)
                     for kt in range(NKT)]
            pts = {}
            stages = {}
            avctx = {}
            hp0_vpairs = []     # vproj0 chunk pairs, emitted in av(0,0,kt)

            def emit_qk_step(idx, drain):
                if idx >= len(STEPS):
                    return
                hp, half, kt = STEPS[idx]
                if half == 0 and kt == 0:
                    drain(f"proj{hp}")      # qraw/kraw for hp must be complete
                qrot, krot = state[hp]
                hoff = half * DH
                ps = psS.tile([P, NQ], f32, tag="s", name="s")
                for qn in range(NQ // 512):
                    nc.tensor.matmul(
                        ps[:, qn * 512:(qn + 1) * 512],
                        krot[hoff:hoff + DH, kt * P:(kt + 1) * P],
                        qrot[hoff:hoff + DH, qn * 512:(qn + 1) * 512],
                        start=True, stop=True)
                pt = ptp.tile([P, NQ], bf16, tag="pt", name="pt")
                nc.scalar.activation(pt[:], ps[:], Exp)
                pts[idx] = pt

            def emit_av_step(idx):
                hp, half, kt = STEPS[idx]
                hoff = half * DH
                if kt == 0:
                    if half == 0:
                        stages[hp] = [astage.tile([P, P], bf16, tag=f"st{qt}",
                                                  name=f"st{qt}")
                                      for qt in range(NQ // P)]
                    banks = [psV.tile([P, 512], f32, tag=f"bank{i}",
                                      name=f"bank{i}") for i in range(2)]
                    for b in banks:
                        nc.vector.memset(b[:], 0.0)
                    avctx[(hp, half)] = banks
                if hp == 0 and half == 0:
                    # vaug[kt] projected in the same window its AV needs it
                    hp0_vpairs[2 * kt]()
                    hp0_vpairs[2 * kt + 1]()
                banks = avctx[(hp, half)]
                pvs = [banks[qt // 4][:, (qt % 4) * P:(qt % 4) * P + 65]
                       for qt in range(NQ // P)]
                pt = pts.pop(idx)
                for qt in range(NQ // P):
                    nc.tensor.matmul(pvs[qt],
                                     pt[:, qt * P:(qt + 1) * P],
                                     vaug[kt][:, hp, half, :],
                                     start=False, stop=(kt == NKT - 1),
                                     skip_group_check=True)
                if kt == NKT - 1:
                    stage = stages[hp]
                    for qt in range(NQ // P):
                        pv = pvs[qt]
                        rec = small.tile([P, 1], f32, tag="rec", name="rec")
                        nc.vector.reciprocal(rec[:], pv[:, 64:65])
                        nc.vector.tensor_scalar_mul(
                            stage[qt][:, hoff:hoff + DH], pv[:, 0:64], rec[:])
                    if half == 1:
                        emit_attn_transpose(hp, stages.pop(hp), banks)
                        avctx.pop((hp, 0))
                        avctx.pop((hp, 1))
                        state.pop(hp)

            def emit_attn_transpose(hp, stage, banks):
                """stage[qt] [tok 128, feat 128] -> attnT[hp] [feat, tok].

                Transposes land in the (just-drained) AV psum banks, viewed
                as bf16, so no extra PSUM bank or psA tag is needed."""
                for qt in range(NQ // P):
                    tr = banks[qt // 4][:, (qt % 4) * P:(qt % 4) * P + 64].bitcast(bf16)
                    nc.tensor.transpose(tr, stage[qt][:], ident[:])
                    nc.vector.tensor_copy(attnT[hp][:, qt * P:(qt + 1) * P], tr)

            def prefetch_wo(n):
                wot = []
                for k in range(KD):
                    t = wstream.tile([P, 512], bf16, tag=f"wo{n}_{k}",
                                     name=f"wo{n}_{k}", bufs=1)
                    nc.sync.dma_start(t[:], wo_d[k * P:(k + 1) * P,
                                                   n * 512:(n + 1) * 512])
                    wot.append(t)
                return wot

            # ---- main loop: attention per (hp, half), with next-hp projection
            # work spliced chunk-by-chunk into the per-kt PE slack ----
            wo_pre = {}
            queue = []          # (label, est_pe_ns, closure) FIFO of deferred work

            def pop_chunk():
                # vproj0 feeds the same-window AV steps of hp0: drain it at
                # double rate so vaug[kt] is ready when av(kt) needs it.
                budget = 520
                while queue and budget > 0:
                    _, est, fn = queue.pop(0)
                    fn()
                    budget -= est

            def drain(label):
                # selective: emit only chunks with this label, preserving
                # their relative order (c1/c2 psA pairs stay adjacent)
                rest = []
                for lb, est, fn in queue:
                    if lb == label:
                        fn()
                    else:
                        rest.append((lb, est, fn))
                queue[:] = rest

            pc0 = proj_chunks(0)        # q(6), k n0/n1 (6), k n2/n3 (6)
            for fn in pc0[:12]:         # needs only xkv cols 0:NQ -- early DMAs
                fn()
            queue.extend(("proj0", est, fn) for est, fn in
                         zip([850, 900, 250] * 2, pc0[12:]))
            vc0 = vproj_chunks(0)
            vc0[0]()                    # wv DMA loads
            hp0_vpairs.extend(vc0[1:])
            LOOKAHEAD = 2
            for i in range(LOOKAHEAD):
                emit_qk_step(i, drain)
            for i, (hp, half, kt) in enumerate(STEPS):
                if half == 0 and kt == 0:
                    if hp + 1 < HPB:
                        queue.extend((f"proj{hp + 1}", est, fn) for est, fn in
                                     zip(proj_chunk_costs(hp + 1),
                                         proj_chunks(hp + 1)))
                    if hp == 1:
                        vc = vproj_chunks(1)
                        queue.extend(
                            ("vproj1", 0 if j == 0 else (850 if j % 2 else 900), fn)
                            for j, fn in enumerate(vc))
                    if hp == HPB - 2:
                        wo_pre[0] = prefetch_wo(0)
                    if hp == HPB - 1:
                        wo_pre[1] = prefetch_wo(1)
                emit_qk_step(i + LOOKAHEAD, drain)
                pop_chunk()     # before AV: hp0's AV needs same-window vproj0
                emit_av_step(i)
            drain("vproj1")

            # ---- out projection: out[tok, DIM] = attnT.T @ Wout ----
            for n in range(DIM // 512):
                wot = wo_pre[n]
                for mt in range(NQ // P):
                    ps = psA.tile([P, 512], f32, tag="ps")
                    for k in range(KD):
                        nc.tensor.matmul(ps[:], attnT[k][:, mt * P:(mt + 1) * P],
                                         wot[k][:],
                                         start=(k == 0), stop=(k == KD - 1))
                    st = ostage.tile([P, 512], f32, tag="ost")
                    nc.vector.tensor_copy(st[:], ps[:])
                    nc.sync.dma_start(
                        out_d[mt * P:(mt + 1) * P, n * 512:(n + 1) * 512], st[:])

    nc.compile()
    return nc


def _prep_inputs(x, sin, cos, Wqkv, Wout):
    """Host-side sharding/layout prep. Returns in_maps list for 8 cores."""
    x = np.asarray(x, np.float32)
    Wqkv = np.asarray(Wqkv, np.float32)
    Wout = np.asarray(Wout, np.float32)
    scale = DH ** -0.5
    wq = (Wqkv[:, :INNER] * scale).astype(BF)
    wk = Wqkv[:, INNER:2 * INNER].astype(BF)
    wv = Wqkv[:, 2 * INNER:].astype(BF)
    wo = Wout.astype(BF)
    cos_pad, sin_pad, Rm = _build_rope_consts(
        np.asarray(sin, np.float32), np.asarray(cos, np.float32))
    rm = Rm.astype(BF)

    in_maps = []
    for c in range(NCORES):
        b, half = divmod(c, 2)
        xT = np.ascontiguousarray(x[b].T)                          # [DIM, N]
        ck, sk = cos_pad, sin_pad
        if half == 1:        # rotate tokens so this core's queries come first
            xT = np.concatenate([xT[:, NQ:], xT[:, :NQ]], axis=1)
            ck = np.concatenate([ck[:, NQ:], ck[:, :NQ]], axis=1)
            sk = np.concatenate([sk[:, NQ:], sk[:, :NQ]], axis=1)
        in_maps.append({
            "xkv": np.ascontiguousarray(xT).astype(BF),
            "wq": wq, "wk": wk, "wv": wv, "wo": wo,
            "cosk": np.ascontiguousarray(ck).astype(BF),
            "sink": np.ascontiguousarray(sk).astype(BF),
            "rm": rm,
            "tid": np.eye(P, dtype=np.float32).astype(BF),
            "par": np.array([[half]], dtype=np.int32),
        })
    return in_maps


LAST_RESULTS = None


def kernel(x, sin, cos, Wqkv, Wout):
    global LAST_RESULTS
    if "nc" not in _CACHE:
        _CACHE["nc"] = _build_program()
    nc = _CACHE["nc"]
    in_maps = _prep_inputs(x, sin, cos, Wqkv, Wout)
    trace = bool(int(os.environ.get("KERNEL_TRACE", "0")))
    try:
        res = run_bass_kernel_spmd(nc, in_maps, core_ids=list(range(NCORES)),
                                   trace=trace)
    except (ImportError, ModuleNotFoundError):
        # NTFF profiling hook unavailable in this environment
        res = run_bass_kernel_spmd(nc, in_maps, core_ids=list(range(NCORES)),
                                   trace=False)
    LAST_RESULTS = res
    out = np.empty((B, N, DIM), np.float32)
    for c in range(NCORES):
        b, half = divmod(c, 2)
        out[b, half * NQ:(half + 1) * NQ, :] = res.results[c]["out"]
    return out

